# revision 23
# baseline (speedup 1.0000x reference)
"""DenseCL loss kernel for 8 TRN2 NeuronCores.

Sharding: core c owns batch image c for the dense branch, queue rows
[c*8192, (c+1)*8192) for the queue-InfoNCE negatives, and the COLUMN block
[c*784, (c+1)*784) of the flat dense-InfoNCE logits.

Key identity: matched_k[j] = k_d[:, idx_j], so the dense logits matrix is a
column gather of P = k_d_local^T @ q_all.  Each core computes partial row
sums Z_i = sum_m c_m * exp(P[m, i] / tau) where c is the histogram of its
own argmax indices (the weighted partition sum runs on the PE with the
counts as a stationary column), and the positives are the sim row maxima.
No matched-key gather and no matched-key AllGather is needed.

Collectives (gpsimd stream): a dummy 32-byte AllGather issued first thing
absorbs the cross-core start-skew barrier; then AllGather of pooled
features, AllGather of normalized q_d (fp8 bytes moved as f32 elements,
hidden under the k branch), AllReduce of the D-sharded global-head
partials.  Final ~10K-flop unshard happens on the host.
"""
import os
import sys

if "/opt/trn_rl_repo" not in sys.path:
    sys.path.insert(0, "/opt/trn_rl_repo")

USE_DR = os.environ.get("KDR", "1") == "1"      # fp8 DoubleRow for dense L1

import numpy as np
import ml_dtypes

import concourse.bass as bass
import concourse.bacc as bacc
import concourse.mybir as mybir
import concourse.tile as tile
from concourse import bass_utils, masks

BF = ml_dtypes.bfloat16
F8NP = ml_dtypes.float8_e4m3
F32 = mybir.dt.float32
BF16 = mybir.dt.bfloat16
F8 = mybir.dt.float8e4
DR = mybir.MatmulPerfMode.DoubleRow

N_CORES = 8
B, HW, C, D, P, Q = 8, 784, 1024, 2048, 128, 65536
QSH = Q // N_CORES          # 8192 queue rows per core
CT, DT = C // 128, D // 128  # 8, 16
GDT = DT // N_CORES         # 2 ghead D-tiles per core
NT = B * HW                 # 6272 total dense rows
TAU = 0.2
LAM = 0.5
ISC = 1.0 / TAU             # 5.0
WSCALE = 32.0               # fp8 range scale for W1/b1 (cancelled by l2 norm)
AF = mybir.ActivationFunctionType
ALU = mybir.AluOpType

# 784 = 6*128 + 16 partition tiles
PT = [(i * 128, min(128, HW - i * 128)) for i in range(7)]
OUTW = 8192                 # out row: [0:6272] Z, 6272 possum,
                            # [6273:6281] qsums, [6281:6289] lpos


def _chunks(n, step=512):
    return [(o, min(step, n - o)) for o in range(0, n, step)]


def _patch_act_tables():
    """Force every activation we use onto the natural_log_exp_and_others
    table set so the kernel needs exactly one ACT_TABLE_LOAD."""
    from concourse import hw_specs
    import concourse.bacc as bacc_mod
    if getattr(bacc_mod, "_act_tables_patched", False):
        return
    orig = hw_specs.get_activation_tables
    ours = {AF.Exp, AF.Ln, AF.Relu, AF.Identity, AF.Copy, AF.Square}
    keep = "natural_log_exp_and_others"

    def patched(arch):
        tabs = orig(arch)
        assert keep in tabs and ours <= tabs[keep]
        return {name: (fns if name == keep else fns - ours)
                for name, fns in tabs.items()}

    bacc_mod.get_activation_tables = patched
    bacc_mod._act_tables_patched = True


def _build(do_compile=True):
    _patch_act_tables()
    nc = bacc.Bacc("TRN2", target_bir_lowering=False, debug=False,
                   num_devices=N_CORES)

    def inp(name, shape, dt):
        return nc.dram_tensor(name, list(shape), dt, kind="ExternalInput")

    xq_d = inp("xq", (128, CT * HW), F8)          # [c, ct*784+p] = feat_q[b, p, ct*128+c]
    xk_d = inp("xk", (128, CT * HW), F8)
    wd1_d = inp("wd1", (DT, 128, C), F8)          # [dt, c, ct*128+d] = 32*Wd1[ct*128+c, dt*128+d]
    wd1m_d = inp("wd1m", (DT, 128, C), F8)
    wd2_d = inp("wd2", (128, D), BF16)            # [c, dt*128+d] = Wd2[dt*128+c, d]
    wd2m_d = inp("wd2m", (128, D), BF16)
    wg1_d = inp("wg1", (128, CT * GDT * 128), BF16)  # per-core D-slice of Wg1
    wg1m_d = inp("wg1m", (128, CT * GDT * 128), BF16)
    wg2_d = inp("wg2", (128, GDT * 128), BF16)    # per-core D-slice of Wg2 (lhsT)
    wg2m_d = inp("wg2m", (128, GDT * 128), BF16)
    bd1_d = inp("bd1", (128, DT), F32)            # [r, dt] = 32*bd1[dt*128+r]
    bd1m_d = inp("bd1m", (128, DT), F32)
    bd2_d = inp("bd2", (128, 1), F32)             # 32*bd2
    bd2m_d = inp("bd2m", (128, 1), F32)
    bg1_d = inp("bg1", (128, GDT), F32)           # per-core D-slice of bg1
    bg1m_d = inp("bg1m", (128, GDT), F32)
    bg2_d = inp("bg2", (128, 1), F32)
    bg2m_d = inp("bg2m", (128, 1), F32)
    queueT_d = inp("queueT", (128, QSH), F8)      # [ch, j] = queue[c0+j, ch]
    iotap_d = inp("iotap", (128, 8), F32)         # col i = p + 128*i
    onesc_d = inp("onesc", (128, 1), F32)         # ones column (lhsT partition sums)
    onesr_d = inp("onesr", (1, 128), F32)         # ones row (lhsT for K=1 broadcast)

    out_d = nc.dram_tensor("out", [1, OUTW], F32, kind="ExternalOutput")

    with tile.TileContext(nc) as tc:
        rg = [list(range(N_CORES))]
        QDW = 128 * HW // 4 + 2 * C   # qd fp8-as-f32 words + pooled feats
        with tc.tile_pool(name="dramp", bufs=1, space="DRAM") as dpool:
            # combined payload: q_d fp8 bytes shipped as f32 elements (4x
            # fewer CCE elements) + the pooled features, one AllGather
            qd_in = dpool.tile([QDW], F32, name="qd_in")
            qd_out = dpool.tile([N_CORES * QDW], F32, name="qd_out",
                                addr_space="Shared")
            gar_in = dpool.tile([128 * 16], F32, name="gar_in")
            gar_out = dpool.tile([128 * 16], F32, name="gar_out",
                                 addr_space="Shared")
            _body(nc, tc, rg, locals())
    if do_compile:
        nc.compile()
    return nc


def _body(nc, tc, rg, env):
    g = lambda k: env[k]

    with tc.tile_pool(name="cst", bufs=1) as cst:

        def load(name, shape, dt, eng=None):
            t = cst.tile(list(shape), dt, name=name + "_sb")
            (eng or nc.sync).dma_start(t[:], g(name + "_d")[:])
            return t

        iotap_sb = load("iotap", (128, 8), F32, eng=nc.gpsimd)
        onesc_sb = load("onesc", (128, 1), F32, eng=nc.gpsimd)
        onesr_sb = load("onesr", (1, 128), F32, eng=nc.gpsimd)

        # ---- sync ring: q-branch critical inputs (per-dt weight slices)
        bd1_sb = load("bd1", (128, DT), F32)
        bd2_sb = load("bd2", (128, 1), F32)
        xq_sb = cst.tile([128, CT * HW], F8, name="xq_sb")
        nc.sync.dma_start(xq_sb[:], g("xq_d")[:])
        wd2_sb = load("wd2", (128, D), BF16)
        wq1_sb = cst.tile([128, DT * C], F8, name="wq1_sb")
        for dt in range(DT):
            nc.sync.dma_start(wq1_sb[:, dt * C:(dt + 1) * C],
                              g("wd1_d")[dt, :, :])
        bd1m_sb = load("bd1m", (128, DT), F32)
        bd2m_sb = load("bd2m", (128, 1), F32)

        # ---- gpsimd (SWDGE) ring: k-branch + tail inputs, so the scalar
        # queue carries only ACT work (DMA triggers head-of-line-block an
        # engine queue once the ring fills)
        xk_sb = cst.tile([128, CT * HW], F8, name="xk_sb")
        nc.gpsimd.dma_start(xk_sb[:], g("xk_d")[:])
        wk1_sb = cst.tile([128, DT * C], F8, name="wk1_sb")
        for dt in range(DT):
            nc.gpsimd.dma_start(wk1_sb[:, dt * C:(dt + 1) * C],
                                g("wd1m_d")[dt, :, :])
        wd2m_sb = load("wd2m", (128, D), BF16, eng=nc.gpsimd)
        with tc.tile_wait_until(0.100):
            queueT_sb = cst.tile([128, QSH], F8, name="queueT_sb")
            nc.sync.dma_start(queueT_sb[:], g("queueT_d")[:])
            bg1_sb = load("bg1", (128, GDT), F32)
            bg1m_sb = load("bg1m", (128, GDT), F32)
            bg2_sb = load("bg2", (128, 1), F32)
            bg2m_sb = load("bg2m", (128, 1), F32)
            wg1_sb = load("wg1", (128, CT * GDT * 128), BF16)
            wg1m_sb = load("wg1m", (128, CT * GDT * 128), BF16)
            wg2_sb = load("wg2", (128, GDT * 128), BF16)
            wg2m_sb = load("wg2m", (128, GDT * 128), BF16)

        onescb_sb = cst.tile([128, 1], BF16, name="onescb_sb")
        nc.vector.tensor_copy(onescb_sb[:], onesc_sb[:])
        id_f = cst.tile([128, 128], F32, name="id_f")
        masks.make_identity(nc, id_f[:])

        # long-lived results
        qdT_bf = cst.tile([128, HW], BF16, name="qdT_bf")
        kdT_bf = cst.tile([128, HW], BF16, name="kdT_bf")
        kdT_f8 = cst.tile([128, HW], F8, name="kdT_f8")
        qdT_f8 = cst.tile([128, HW], F8, name="qdT_f8")
        qall_sb = cst.tile([128, NT], F8, name="qall_sb")
        qgT_bf = cst.tile([128, 8], BF16, name="qgT_bf")
        kgT_bf = cst.tile([128, 8], BF16, name="kgT_bf")
        qgT_f8 = cst.tile([128, 8], F8, name="qgT_f8")
        pool_sb = cst.tile([128, 16], F32, name="pool_sb")
        gqall = cst.tile([128, 64], F32, name="gqall")  # pooled q [c, (r t)]
        gkall = cst.tile([128, 64], F32, name="gkall")
        cpartb = cst.tile([128, 7], BF16, name="cpartb")  # histogram counts
        fin_sb = cst.tile([1, 16], F32, name="fin_sb")

        _dense(nc, tc, rg, env, cst, locals())
        _ghead(nc, tc, rg, env, cst, locals())
        _simhist(nc, tc, rg, env, cst, locals())
        _logits(nc, tc, rg, env, cst, locals())


def _dense(nc, tc, rg, env, cst, ctx):
    g = lambda k: env[k]
    c = lambda k: ctx[k]
    qd_in, qd_out = g("qd_in"), g("qd_out")
    QDP = 128 * HW // 4
    xq_sb, xk_sb = c("xq_sb"), c("xk_sb")
    pool_sb = c("pool_sb")

    with tc.tile_pool(name="hp", bufs=3) as hp, \
         tc.tile_pool(name="l2s", bufs=2) as l2s, \
         tc.tile_pool(name="plp", bufs=2) as plp, \
         tc.tile_pool(name="ps_big", bufs=2, space="PSUM") as ps_big, \
         tc.tile_pool(name="ps_qd", bufs=2, space="PSUM") as ps_qd:

        def dense_branch(br, xs, w1sb, w2sb, b1, b2, dst, dst8):
            qd_ps = ps_qd.tile([128, HW], F32, name="qd_ps", tag="qd")
            for dt in range(DT):
                w1t = w1sb[:, dt * C:(dt + 1) * C]
                h_ps = ps_big.tile([128, HW], F32, name="h_ps", tag="big")
                if USE_DR:
                    for cp in range(CT // 2):
                        wp = w1t[:, cp * 256:(cp + 1) * 256].rearrange(
                            "p (two m) -> p two m", two=2)
                        xp = xs[:, cp * 2 * HW:(cp + 1) * 2 * HW].rearrange(
                            "p (two n) -> p two n", two=2)
                        for (o, n) in _chunks(HW):
                            nc.tensor.matmul(
                                h_ps[:, o:o + n],
                                lhsT=wp,
                                rhs=xp[:, :, o:o + n],
                                start=(cp == 0), stop=(cp == CT // 2 - 1),
                                perf_mode=DR)
                else:
                    for ct in range(CT):
                        for (o, n) in _chunks(HW):
                            nc.tensor.matmul(
                                h_ps[:, o:o + n],
                                lhsT=w1t[:, ct * 128:(ct + 1) * 128],
                                rhs=xs[:, ct * HW + o:ct * HW + o + n],
                                start=(ct == 0), stop=(ct == CT - 1))
                h_sb = hp.tile([128, HW], BF16, name="h_sb")
                nc.scalar.activation(h_sb[:], h_ps[:], AF.Relu,
                                     bias=b1[:, dt:dt + 1])
                if br == 0 and dt == 9:
                    pin = qd_in[QDP:QDP + 2 * C].rearrange(
                        "(g t c) -> c (g t)", g=2, t=8, c=128)
                    nc.gpsimd.dma_start(pin, pool_sb[:])
                if br == 0 and dt in (2, 4, 6, 8):
                    # pooling of xq/xk on DVE while PE grinds L1
                    base = 0 if dt in (2, 4) else 8
                    src = xq_sb if dt in (2, 4) else xk_sb
                    c0 = 0 if dt in (2, 6) else 4
                    for ct2 in range(c0, c0 + 4):
                        scr = plp.tile([128, HW], BF16, name="pool_scr")
                        nc.vector.tensor_scalar(
                            scr[:], src[:, ct2 * HW:(ct2 + 1) * HW], 1.0,
                            None, op0=ALU.mult, op1=ALU.add,
                            accum_out=pool_sb[:, base + ct2:base + ct2 + 1])
                for (o, n) in _chunks(HW):
                    nc.tensor.matmul(
                        qd_ps[:, o:o + n],
                        lhsT=w2sb[:, dt * 128:(dt + 1) * 128],
                        rhs=h_sb[:, o:o + n],
                        start=(dt == 0), stop=(dt == DT - 1))
            # bias + l2-normalize along channels (partition dim)
            qdT_f = l2s.tile([128, HW], F32, name="qdT_f")
            nc.scalar.activation(qdT_f[:], qd_ps[:], AF.Identity, bias=b2[:])
            sq = l2s.tile([128, HW], BF16, name="sq")
            nc.scalar.activation(sq[:], qdT_f[:], AF.Square)
            ssq_ps = ps_qd.tile([1, HW], F32, name="ssq_ps", tag="qd")
            for (o, n) in _chunks(HW):
                nc.tensor.matmul(ssq_ps[:, o:o + n], lhsT=c("onescb_sb")[:],
                                 rhs=sq[:, o:o + n], start=True, stop=True)
            nrm = l2s.tile([1, HW], F32, name="nrm")
            nc.vector.tensor_scalar_max(nrm[:], ssq_ps[:], 1e-12)
            # rsqrt(s) = exp(-0.5*ln(s)) keeps ACT on one table set
            nrm2 = l2s.tile([1, HW], F32, name="nrm2")
            nc.scalar.activation(nrm2[:], nrm[:], AF.Ln)
            rn = l2s.tile([1, HW], F32, name="rn")
            nc.scalar.activation(rn[:], nrm2[:], AF.Exp, scale=-0.5)
            rnb_ps = ps_qd.tile([128, HW], F32, name="rnb_ps", tag="qd")
            for (o, n) in _chunks(HW):
                nc.tensor.matmul(rnb_ps[:, o:o + n], lhsT=c("onesr_sb")[:],
                                 rhs=rn[:, o:o + n], start=True, stop=True)
            nc.vector.tensor_mul(dst[:], qdT_f[:], rnb_ps[:])
            nc.vector.tensor_copy(dst8[:], dst[:])

        dense_branch(0, xq_sb, c("wq1_sb"), c("wd2_sb"), c("bd1_sb"),
                     c("bd2_sb"), c("qdT_bf"), c("qdT_f8"))
        # ship q_d + pooled feats in ONE AllGather: this collective gates
        # the whole logits tail, so it goes first on the collective stream.
        nc.sync.dma_start(
            qd_in[0:QDP].rearrange("(c p) -> c p", c=128),
            c("qdT_f8")[:].bitcast(F32))
        nc.gpsimd.collective_compute(
            "AllGather", ALU.bypass, replica_groups=rg,
            ins=[qd_in.opt()], outs=[qd_out.opt()])
        qov = qd_out[:].rearrange("(r x) -> r x", r=8)
        nc.sync.dma_start(
            c("qall_sb")[:].bitcast(F32).rearrange("c (r p) -> c r p", r=8),
            qov[:, 0:QDP].rearrange("r (c p) -> c r p", c=128))
        # pooled features for every image: [c, (r t)] layout (deferred so
        # these loads never head-of-line-block the qall load on sync)
        with tc.tile_wait_until(0.095):
            for gi, dstp in ((0, c("gqall")), (1, c("gkall"))):
                for r in range(8):
                    nc.sync.dma_start(
                        dstp[:, r * 8:(r + 1) * 8],
                        qov[r, QDP + gi * C:QDP + (gi + 1) * C].rearrange(
                            "(t c) -> c t", c=128))

        dense_branch(1, xk_sb, c("wk1_sb"), c("wd2m_sb"), c("bd1m_sb"),
                     c("bd2m_sb"), c("kdT_bf"), c("kdT_f8"))


def _ghead(nc, tc, rg, env, cst, ctx):
    """D-sharded global heads (2 of 16 D-tiles per core) + AllReduce."""
    g = lambda k: env[k]
    c = lambda k: ctx[k]

    with tc.tile_wait_until(0.100), \
         tc.tile_pool(name="gh", bufs=1) as gh, \
         tc.tile_pool(name="ps_gh", bufs=2, space="PSUM") as ps_gh, \
         tc.tile_pool(name="ps_gq", bufs=1, space="PSUM") as ps_gq:
        gq_bf = gh.tile([128, 64], BF16, name="gq_bf")
        gk_bf = gh.tile([128, 64], BF16, name="gk_bf")
        nc.vector.tensor_scalar_mul(gq_bf[:], c("gqall")[:], 1.0 / HW)
        nc.vector.tensor_scalar_mul(gk_bf[:], c("gkall")[:], 1.0 / HW)
        gprt = gh.tile([128, 16], F32, name="gprt")
        for br2, (gsb, w1sb, w2sb, b1c) in enumerate([
                (gq_bf, c("wg1_sb"), c("wg2_sb"), c("bg1_sb")),
                (gk_bf, c("wg1m_sb"), c("wg2m_sb"), c("bg1m_sb"))]):
            gv = gsb[:].rearrange("c (r t) -> c t r", t=8)
            qg_ps = ps_gq.tile([128, 8], F32, name="qg_ps", tag="qg")
            for dl in range(GDT):
                hgt_ps = ps_gh.tile([128, 8], F32, name="hgt_ps")
                for ct in range(CT):
                    nc.tensor.matmul(
                        hgt_ps[:],
                        lhsT=w1sb[:, (ct * GDT + dl) * 128:
                                  (ct * GDT + dl + 1) * 128],
                        rhs=gv[:, ct, :],
                        start=(ct == 0), stop=(ct == CT - 1))
                hgt_sb = gh.tile([128, 8], BF16, name=f"hgt{br2}_{dl}")
                nc.vector.tensor_scalar(hgt_sb[:], hgt_ps[:],
                                        b1c[:, dl:dl + 1], 0.0,
                                        op0=ALU.add, op1=ALU.max)
                nc.tensor.matmul(qg_ps[:],
                                 lhsT=w2sb[:, dl * 128:(dl + 1) * 128],
                                 rhs=hgt_sb[:], start=(dl == 0),
                                 stop=(dl == GDT - 1))
            nc.vector.tensor_copy(gprt[:, br2 * 8:br2 * 8 + 8], qg_ps[:])
        nc.gpsimd.dma_start(
            g("gar_in")[:].rearrange("(c p) -> c p", c=128), gprt[:])
        nc.gpsimd.collective_compute(
            "AllReduce", ALU.add, replica_groups=rg,
            ins=[g("gar_in").opt()], outs=[g("gar_out").opt()])


def _simhist(nc, tc, rg, env, cst, ctx):
    """sim + argmax + histogram of matched indices + positives partial."""
    g = lambda k: env[k]
    c = lambda k: ctx[k]
    out_d = g("out_d")
    qdT_bf, kdT_bf = c("qdT_bf"), c("kdT_bf")
    fin_sb = c("fin_sb")

    with tc.tile_pool(name="cor", bufs=1) as cor, \
         tc.tile_pool(name="corS", bufs=2) as corS, \
         tc.tile_pool(name="ps_sim", bufs=2, space="PSUM") as ps_sim, \
         tc.tile_pool(name="ps_ir", bufs=2, space="PSUM") as ps_ir:
        sim_sb = cor.tile([128, 7 * HW], BF16, name="sim_sb")
        mx8 = cor.tile([128, 8], BF16, name="mx8")
        ix8 = cor.tile([128, 8], mybir.dt.uint32, name="ix8")
        ixf = cor.tile([128, 7], F32, name="ixf")
        posv = cor.tile([128, 7], F32, name="posv")
        nc.vector.memset(posv[:], 0.0)
        for i, (po_, pn) in enumerate(PT):
            s_ps = ps_sim.tile([128, HW], F32, name="s_ps", tag="sim")
            for (o, n) in _chunks(HW):
                nc.tensor.matmul(s_ps[0:pn, o:o + n],
                                 lhsT=qdT_bf[:, po_:po_ + pn],
                                 rhs=kdT_bf[:, o:o + n],
                                 start=True, stop=True)
            nc.scalar.activation(sim_sb[0:pn, i * HW:i * HW + HW],
                                 s_ps[0:pn, :], AF.Copy)
            nc.vector.max(mx8[0:pn, :], sim_sb[0:pn, i * HW:i * HW + HW])
            nc.vector.max_index(ix8[0:pn, :], mx8[0:pn, :],
                                sim_sb[0:pn, i * HW:i * HW + HW])
            nc.vector.tensor_copy(ixf[0:pn, i:i + 1], ix8[0:pn, 0:1])
            nc.vector.tensor_copy(posv[0:pn, i:i + 1], mx8[0:pn, 0:1])
        # broadcast idx row to all partitions
        ir_sb = cor.tile([1, HW], F32, name="ir_sb")
        for i, (po_, pn) in enumerate(PT):
            ir_ps = ps_ir.tile([1, 128], F32, name="ir_ps", tag="ir")
            nc.tensor.transpose(ir_ps[0:1, 0:pn], ixf[0:pn, i:i + 1],
                                c("id_f")[0:pn, 0:pn])
            nc.scalar.activation(ir_sb[0:1, po_:po_ + pn],
                                 ir_ps[0:1, 0:pn], AF.Copy)
        ib_ps = ps_sim.tile([128, HW], F32, name="ib_ps", tag="sim")
        for (o, n) in _chunks(HW):
            nc.tensor.matmul(ib_ps[:, o:o + n], lhsT=c("onesr_sb")[:],
                             rhs=ir_sb[:, o:o + n], start=True, stop=True)
        ib_sb = cor.tile([128, HW], F32, name="ib_sb")
        nc.scalar.activation(ib_sb[:], ib_ps[:], AF.Copy)
        # histogram: count idx == m via is_equal + free-axis accumulate
        cpart = cor.tile([128, 7], F32, name="cpart")
        nc.vector.memset(cpart[:], 0.0)
        for i, (po_, pn) in enumerate(PT):
            S = corS.tile([128, HW], BF16, name="S")
            nc.vector.tensor_scalar(
                S[0:pn, :], ib_sb[0:pn, :], c("iotap_sb")[0:pn, i:i + 1],
                None, op0=ALU.is_equal, op1=ALU.add,
                accum_out=cpart[0:pn, i:i + 1])
        nc.vector.tensor_copy(c("cpartb")[:], cpart[:])
        # positives partial: sum(max sim) over own rows
        pos_ps = ps_ir.tile([1, 128], F32, name="pos_ps", tag="ir")
        nc.tensor.matmul(pos_ps[0:1, 0:7], lhsT=c("onesc_sb")[:],
                         rhs=posv[:], start=True, stop=True)
        nc.vector.reduce_sum(fin_sb[0:1, 0:1], pos_ps[0:1, 0:7],
                             axis=mybir.AxisListType.X)
    nc.sync.dma_start(out_d[0:1, NT:NT + 1], fin_sb[0:1, 0:1])


def _logits(nc, tc, rg, env, cst, ctx):
    """Column-sharded dense-InfoNCE partial Z + qg norm/lpos/queue negs.

    [m, i] orientation: P^T tiles (lhsT = k_d m-tile, rhs = q_all), exp to
    E' in SBUF, then Z[i] = sum_m c_m E'[m, i] on the PE with the counts
    column as the stationary operand.
    """
    g = lambda k: env[k]
    c = lambda k: ctx[k]
    out_d = g("out_d")
    gar_out = g("gar_out")
    qall_sb, kdT_f8 = c("qall_sb"), c("kdT_f8")
    fin_sb, cpartb = c("fin_sb"), c("cpartb")
    qgT_bf, kgT_bf, qgT_f8 = c("qgT_bf"), c("kgT_bf"), c("qgT_f8")

    with tc.tile_pool(name="lg", bufs=1) as lgp, \
         tc.tile_pool(name="gn", bufs=1) as gn, \
         tc.tile_pool(name="ps_lg", bufs=2, space="PSUM") as ps_lg, \
         tc.tile_pool(name="ps_z", bufs=2, space="PSUM") as ps_z, \
         tc.tile_pool(name="ps_gn", bufs=1, space="PSUM") as ps_gn:
        E_all = lgp.tile([128, 7 * NT], BF16, name="E_all")
        z_row = lgp.tile([1, NT], F32, name="z_row")
        # P^T + exp, i-chunk-major so the Z sums can chase the exp chain
        for (co, cn) in _chunks(NT, 1024):
            for j, (po_, pn) in enumerate(PT):
                lg_ps = ps_lg.tile([128, 1024], F32, name="lg_ps")
                for (o, n) in _chunks(cn):
                    nc.tensor.matmul(
                        lg_ps[0:pn, o:o + n],
                        lhsT=kdT_f8[:, po_:po_ + pn],
                        rhs=qall_sb[:, co + o:co + o + n],
                        start=True, stop=True)
                nc.scalar.activation(E_all[0:pn, j * NT + co:j * NT + co + cn],
                                     lg_ps[0:pn, 0:cn], AF.Exp, scale=ISC)

        # ---- Z[i] = sum_m c_m E'[m, i] on the PE, chasing the exp chain
        for (co, cn) in _chunks(NT):
            zc_ps = ps_z.tile([1, 512], F32, name="zc_ps", tag="z")
            for j, (po_, pn) in enumerate(PT):
                nc.tensor.matmul(zc_ps[0:1, 0:cn],
                                 lhsT=cpartb[0:pn, j:j + 1],
                                 rhs=E_all[0:pn, j * NT + co:j * NT + co + cn],
                                 start=(j == 0), stop=(j == 6))
            nc.scalar.activation(z_row[0:1, co:co + cn], zc_ps[0:1, 0:cn],
                                 AF.Copy)
        nc.sync.dma_start(out_d[0:1, 0:NT], z_row[:])

        # ---- qg/kg bias + l2 norm + lpos + queue negatives (gated on the
        # ghead AllReduce; scheduled late so they never stall the exp chain)
        with tc.tile_wait_until(0.115):
            garT = gn.tile([128, 16], F32, name="garT")
            nc.gpsimd.dma_start(
                garT[:], gar_out[:].rearrange("(c p) -> c p", c=128))
            for br2, (b2, dstg) in enumerate([(c("bg2_sb"), qgT_bf),
                                              (c("bg2m_sb"), kgT_bf)]):
                qgT_f = gn.tile([128, 8], F32, name=f"qgT_f{br2}")
                nc.scalar.activation(qgT_f[:], garT[:, br2 * 8:br2 * 8 + 8],
                                     AF.Identity, bias=b2[:])
                sqg = gn.tile([128, 8], BF16, name=f"sqg{br2}")
                nc.scalar.activation(sqg[:], qgT_f[:], AF.Square)
                ssg_ps = ps_gn.tile([1, 8], F32, name="ssg_ps", tag="g8")
                nc.tensor.matmul(ssg_ps[:], lhsT=c("onescb_sb")[:],
                                 rhs=sqg[:], start=True, stop=True)
                nrg = gn.tile([1, 8], F32, name=f"nrg{br2}")
                nc.vector.tensor_scalar_max(nrg[:], ssg_ps[:], 1e-12)
                nrg2 = gn.tile([1, 8], F32, name=f"nrg2{br2}")
                nc.scalar.activation(nrg2[:], nrg[:], AF.Ln)
                rng = gn.tile([1, 8], F32, name=f"rng{br2}")
                nc.scalar.activation(rng[:], nrg2[:], AF.Exp, scale=-0.5)
                rngb_ps = ps_gn.tile([128, 8], F32, name="rngb_ps", tag="g8")
                nc.tensor.matmul(rngb_ps[:], lhsT=c("onesr_sb")[:],
                                 rhs=rng[:], start=True, stop=True)
                nc.vector.tensor_mul(dstg[:], qgT_f[:], rngb_ps[:])
            lpm = gn.tile([128, 8], F32, name="lpm")
            nc.vector.tensor_mul(lpm[:], qgT_bf[:], kgT_bf[:])
            lp_ps = ps_gn.tile([1, 8], F32, name="lp_ps", tag="g8")
            nc.tensor.matmul(lp_ps[:], lhsT=c("onesc_sb")[:], rhs=lpm[:],
                             start=True, stop=True)
            nc.scalar.activation(fin_sb[0:1, 1:9], lp_ps[:], AF.Copy)
            nc.sync.dma_start(out_d[0:1, NT + 9:NT + 17], fin_sb[0:1, 1:9])
            nc.vector.tensor_copy(qgT_f8[:], qgT_bf[:])
            # queue negatives: transposed orientation, single exp
            qe_ps = ps_z.tile([128, 512], F32, name="qe_ps", tag="z")
            for qt in range(64):
                nc.tensor.matmul(
                    qe_ps[:, qt * 8:(qt + 1) * 8],
                    lhsT=c("queueT_sb")[:, qt * 128:(qt + 1) * 128],
                    rhs=qgT_f8[:], start=True, stop=True)
            qe_sb = gn.tile([128, 512], BF16, name="qe_sb")
            nc.scalar.activation(qe_sb[:], qe_ps[:], AF.Exp, scale=ISC)
            qs_ps = ps_gn.tile([1, 512], F32, name="qs_ps", tag="g8")
            for (o, n) in _chunks(512):
                nc.tensor.matmul(qs_ps[:, o:o + n], lhsT=c("onescb_sb")[:],
                                 rhs=qe_sb[:, o:o + n], start=True, stop=True)
            qsum_r = gn.tile([1, 8], F32, name="qsum_r")
            nc.vector.reduce_sum(qsum_r[:],
                                 qs_ps[:].rearrange("p (t i) -> p i t", i=8),
                                 axis=mybir.AxisListType.X)
            nc.sync.dma_start(out_d[0:1, NT + 1:NT + 9], qsum_r[:])



def _prep_inputs(inputs):
    fq = np.asarray(inputs["feat_q"], np.float32).reshape(B, HW, C)
    fk = np.asarray(inputs["feat_k"], np.float32).reshape(B, HW, C)

    def xT(x):  # (784, 1024) -> (128, 8*784) fp8 with [c, ct*784+p]
        return np.ascontiguousarray(
            x.reshape(HW, CT, 128).transpose(2, 1, 0).reshape(128, CT * HW)
        ).astype(F8NP)

    def w1tile(w):  # (1024, 2048) -> (16, 128, 1024) fp8, scaled
        return np.ascontiguousarray(
            (w * WSCALE).reshape(CT, 128, DT, 128).transpose(2, 1, 0, 3)
            .reshape(DT, 128, C)).astype(F8NP)

    def w2tile(w):  # (2048, 128) -> (128, 2048) with [c, dt*128+d]
        return np.ascontiguousarray(
            w.reshape(DT, 128, 128).transpose(1, 0, 2).reshape(128, D)
        ).astype(BF)

    queue = np.asarray(inputs["queue"], np.float32)
    wg1 = np.asarray(inputs["Wg1"], np.float32)   # (1024, 2048)
    wg1m = np.asarray(inputs["mWg1"], np.float32)
    wg2 = np.asarray(inputs["Wg2"], np.float32)   # (2048, 128)
    wg2m = np.asarray(inputs["mWg2"], np.float32)
    bg1 = np.asarray(inputs["bg1"], np.float32)
    bg1m = np.asarray(inputs["mbg1"], np.float32)

    iotap = (np.arange(128, dtype=np.float32)[:, None]
             + 128.0 * np.arange(8, dtype=np.float32)[None, :])

    shared = {
        "wd1": w1tile(np.asarray(inputs["Wd1"], np.float32)),
        "wd1m": w1tile(np.asarray(inputs["mWd1"], np.float32)),
        "wd2": w2tile(np.asarray(inputs["Wd2"], np.float32)),
        "wd2m": w2tile(np.asarray(inputs["mWd2"], np.float32)),
        "bd1": np.ascontiguousarray(
            (np.asarray(inputs["bd1"], np.float32) * WSCALE)
            .reshape(DT, 128).T).astype(np.float32),
        "bd1m": np.ascontiguousarray(
            (np.asarray(inputs["mbd1"], np.float32) * WSCALE)
            .reshape(DT, 128).T).astype(np.float32),
        "bd2": (np.asarray(inputs["bd2"], np.float32) * WSCALE
                ).reshape(128, 1),
        "bd2m": (np.asarray(inputs["mbd2"], np.float32) * WSCALE
                 ).reshape(128, 1),
        "bg2": np.asarray(inputs["bg2"], np.float32).reshape(128, 1),
        "bg2m": np.asarray(inputs["mbg2"], np.float32).reshape(128, 1),
        "iotap": np.ascontiguousarray(iotap),
        "onesc": np.ones((128, 1), np.float32),
        "onesr": np.ones((1, 128), np.float32),
    }
    in_maps = []
    for cc in range(N_CORES):
        m = dict(shared)
        m["xq"] = xT(fq[cc])
        m["xk"] = xT(fk[cc])
        m["queueT"] = np.ascontiguousarray(
            queue[cc * QSH:(cc + 1) * QSH].T).astype(F8NP)
        # per-core D-slice of the global head: dts {2c, 2c+1}
        dsl = slice(cc * GDT * 128, (cc + 1) * GDT * 128)
        # wg1 slice layout [c, (ct*GDT+dl)*128 + d]
        m["wg1"] = np.ascontiguousarray(
            wg1[:, dsl].reshape(CT, 128, GDT * 128).transpose(1, 0, 2)
            .reshape(128, CT * GDT * 128)).astype(BF)
        m["wg1m"] = np.ascontiguousarray(
            wg1m[:, dsl].reshape(CT, 128, GDT * 128).transpose(1, 0, 2)
            .reshape(128, CT * GDT * 128)).astype(BF)
        # wg2 slice [dl*128+r, P] -> lhsT layout [r, dl*128+p]
        m["wg2"] = np.ascontiguousarray(
            wg2[dsl].reshape(GDT, 128, 128).transpose(1, 0, 2)
            .reshape(128, GDT * 128)).astype(BF)
        m["wg2m"] = np.ascontiguousarray(
            wg2m[dsl].reshape(GDT, 128, 128).transpose(1, 0, 2)
            .reshape(128, GDT * 128)).astype(BF)
        m["bg1"] = np.ascontiguousarray(
            bg1[dsl].reshape(GDT, 128).T).astype(np.float32)
        m["bg1m"] = np.ascontiguousarray(
            bg1m[dsl].reshape(GDT, 128).T).astype(np.float32)
        in_maps.append(m)
    return in_maps


_NC = None


def _get_nc():
    global _NC
    if _NC is None:
        _NC = _build()
    return _NC


def _host_combine(outs):
    """outs: [8, 1, OUTW] per-core partial rows -> final scalar loss.

    Per core: [0:6272] Z row-sum partials over its 784 logit columns,
    [6272] partial sum(max sim) over its rows, [6273:6281] partial
    sum(exp(l_neg/tau)) per image over its queue shard, [6281:6289]
    l_pos per image (replicated).
    """
    outs = np.asarray(outs, np.float64).reshape(len(outs), -1)
    Zf = outs[:, 0:NT].sum(axis=0)
    possum = outs[:, NT].sum()
    l_d = np.mean(np.log(Zf)) - ISC * possum / NT
    qsums = outs[:, NT + 1:NT + 9].sum(axis=0)
    lpos = outs[0, NT + 9:NT + 17]
    lse = np.log(np.exp(ISC * lpos) + qsums)
    l_g = np.mean(lse - ISC * lpos)
    return np.float32((1.0 - LAM) * l_g + LAM * l_d).reshape(())


def kernel(**inputs) -> np.ndarray:
    nc = _get_nc()
    in_maps = _prep_inputs(inputs)
    res = bass_utils.run_bass_kernel_spmd(nc, in_maps,
                                          core_ids=list(range(N_CORES)))
    outs = np.stack([res.results[c]["out"].reshape(1, OUTW)
                     for c in range(N_CORES)])
    return _host_combine(outs)


# revision 26
# speedup vs baseline: 1.0503x; 1.0503x over previous
"""DenseCL loss kernel for 8 TRN2 NeuronCores.

Sharding: core c owns batch image c for the dense branch, queue rows
[c*8192, (c+1)*8192) for the queue-InfoNCE negatives, and the COLUMN block
[c*784, (c+1)*784) of the flat dense-InfoNCE logits.

Key identity: matched_k[j] = k_d[:, idx_j], so the dense logits matrix is a
column gather of P = k_d_local^T @ q_all.  Each core computes partial row
sums Z_i = sum_m c_m * exp(P[m, i] / tau) where c is the histogram of its
own argmax indices (the weighted partition sum runs on the PE with the
counts as a stationary column), and the positives are the sim row maxima.
No matched-key gather and no matched-key AllGather is needed.

Collectives (gpsimd stream): a dummy 32-byte AllGather issued first thing
absorbs the cross-core start-skew barrier; then AllGather of pooled
features, AllGather of normalized q_d (fp8 bytes moved as f32 elements,
hidden under the k branch), AllReduce of the D-sharded global-head
partials.  Final ~10K-flop unshard happens on the host.
"""
import os
import sys

if "/opt/trn_rl_repo" not in sys.path:
    sys.path.insert(0, "/opt/trn_rl_repo")

USE_DR = os.environ.get("KDR", "1") == "1"      # fp8 DoubleRow for dense L1

import numpy as np
import ml_dtypes

import concourse.bass as bass
import concourse.bacc as bacc
import concourse.mybir as mybir
import concourse.tile as tile
from concourse import bass_utils, masks

BF = ml_dtypes.bfloat16
F8NP = ml_dtypes.float8_e4m3
F32 = mybir.dt.float32
BF16 = mybir.dt.bfloat16
F8 = mybir.dt.float8e4
DR = mybir.MatmulPerfMode.DoubleRow

N_CORES = 8
B, HW, C, D, P, Q = 8, 784, 1024, 2048, 128, 65536
QSH = Q // N_CORES          # 8192 queue rows per core
CT, DT = C // 128, D // 128  # 8, 16
GDT = DT // N_CORES         # 2 ghead D-tiles per core
NT = B * HW                 # 6272 total dense rows
TAU = 0.2
LAM = 0.5
ISC = 1.0 / TAU             # 5.0
WSCALE = 32.0               # fp8 range scale for W1/b1 (cancelled by l2 norm)
AF = mybir.ActivationFunctionType
ALU = mybir.AluOpType

# 784 = 6*128 + 16 partition tiles
PT = [(i * 128, min(128, HW - i * 128)) for i in range(7)]
OUTW = 8192                 # out row: [0:6272] Z, 6272 possum,
                            # [6273:6281] qsums, [6281:6289] lpos


def _chunks(n, step=512):
    return [(o, min(step, n - o)) for o in range(0, n, step)]


def _patch_act_tables():
    """Force every activation we use onto the natural_log_exp_and_others
    table set so the kernel needs exactly one ACT_TABLE_LOAD."""
    from concourse import hw_specs
    import concourse.bacc as bacc_mod
    if getattr(bacc_mod, "_act_tables_patched", False):
        return
    orig = hw_specs.get_activation_tables
    ours = {AF.Exp, AF.Ln, AF.Relu, AF.Identity, AF.Copy, AF.Square}
    keep = "natural_log_exp_and_others"

    def patched(arch):
        tabs = orig(arch)
        assert keep in tabs and ours <= tabs[keep]
        return {name: (fns if name == keep else fns - ours)
                for name, fns in tabs.items()}

    bacc_mod.get_activation_tables = patched
    bacc_mod._act_tables_patched = True


def _build(do_compile=True):
    _patch_act_tables()
    nc = bacc.Bacc("TRN2", target_bir_lowering=False, debug=False,
                   num_devices=N_CORES)

    def inp(name, shape, dt):
        return nc.dram_tensor(name, list(shape), dt, kind="ExternalInput")

    xq_d = inp("xq", (128, CT * HW), F8)          # [c, ct*784+p] = feat_q[b, p, ct*128+c]
    xk_d = inp("xk", (128, CT * HW), F8)
    wd1_d = inp("wd1", (DT, 128, C), F8)          # [dt, c, ct*128+d] = 32*Wd1[ct*128+c, dt*128+d]
    wd1m_d = inp("wd1m", (DT, 128, C), F8)
    wd2_d = inp("wd2", (128, D), BF16)            # [c, dt*128+d] = Wd2[dt*128+c, d]
    wd2m_d = inp("wd2m", (128, D), BF16)
    wg1_d = inp("wg1", (128, CT * GDT * 128), BF16)  # per-core D-slice of Wg1
    wg1m_d = inp("wg1m", (128, CT * GDT * 128), BF16)
    wg2_d = inp("wg2", (128, GDT * 128), BF16)    # per-core D-slice of Wg2 (lhsT)
    wg2m_d = inp("wg2m", (128, GDT * 128), BF16)
    bd1_d = inp("bd1", (128, DT), F32)            # [r, dt] = 32*bd1[dt*128+r]
    bd1m_d = inp("bd1m", (128, DT), F32)
    bd2_d = inp("bd2", (128, 1), F32)             # 32*bd2
    bd2m_d = inp("bd2m", (128, 1), F32)
    bg1_d = inp("bg1", (128, GDT), F32)           # per-core D-slice of bg1
    bg1m_d = inp("bg1m", (128, GDT), F32)
    bg2_d = inp("bg2", (128, 1), F32)
    bg2m_d = inp("bg2m", (128, 1), F32)
    queueT_d = inp("queueT", (128, QSH), F8)      # [ch, j] = queue[c0+j, ch]
    iotap_d = inp("iotap", (128, 8), F32)         # col i = p + 128*i
    onesc_d = inp("onesc", (128, 1), F32)         # ones column (lhsT partition sums)
    onesr_d = inp("onesr", (1, 128), F32)         # ones row (lhsT for K=1 broadcast)

    out_d = nc.dram_tensor("out", [1, OUTW], F32, kind="ExternalOutput")

    with tile.TileContext(nc) as tc:
        rg = [list(range(N_CORES))]
        with tc.tile_pool(name="dramp", bufs=1, space="DRAM") as dpool:
            pool_in = dpool.tile([2 * C], F32, name="pool_in")
            pool_out = dpool.tile([N_CORES * 2 * C], F32, name="pool_out",
                                  addr_space="Shared")
            # q_d fp8 bytes shipped as f32 elements (4x fewer CCE elements)
            qd_in = dpool.tile([128 * HW // 4], F32, name="qd_in")
            qd_out = dpool.tile([N_CORES * 128 * HW // 4], F32, name="qd_out",
                                addr_space="Shared")
            gar_in = dpool.tile([128 * 16], F32, name="gar_in")
            gar_out = dpool.tile([128 * 16], F32, name="gar_out",
                                 addr_space="Shared")
            _body(nc, tc, rg, locals())
    if do_compile:
        nc.compile()
    return nc


def _body(nc, tc, rg, env):
    g = lambda k: env[k]

    with tc.tile_pool(name="cst", bufs=1) as cst:

        def load(name, shape, dt, eng=None):
            t = cst.tile(list(shape), dt, name=name + "_sb")
            (eng or nc.sync).dma_start(t[:], g(name + "_d")[:])
            return t

        iotap_sb = load("iotap", (128, 8), F32, eng=nc.gpsimd)
        onesc_sb = load("onesc", (128, 1), F32, eng=nc.gpsimd)
        onesr_sb = load("onesr", (1, 128), F32, eng=nc.gpsimd)

        # ---- sync ring: q-branch critical inputs (per-dt weight slices)
        bd1_sb = load("bd1", (128, DT), F32)
        bd2_sb = load("bd2", (128, 1), F32)
        xq_sb = cst.tile([128, CT * HW], F8, name="xq_sb")
        nc.sync.dma_start(xq_sb[:], g("xq_d")[:])
        wd2_sb = load("wd2", (128, D), BF16)
        wq1_sb = cst.tile([128, DT * C], F8, name="wq1_sb")
        for dt in range(DT):
            nc.sync.dma_start(wq1_sb[:, dt * C:(dt + 1) * C],
                              g("wd1_d")[dt, :, :])
        bd1m_sb = load("bd1m", (128, DT), F32)
        bd2m_sb = load("bd2m", (128, 1), F32)

        # ---- gpsimd (SWDGE) ring: k-branch + tail inputs, so the scalar
        # queue carries only ACT work (DMA triggers head-of-line-block an
        # engine queue once the ring fills)
        xk_sb = cst.tile([128, CT * HW], F8, name="xk_sb")
        nc.gpsimd.dma_start(xk_sb[:], g("xk_d")[:])
        wk1_sb = cst.tile([128, DT * C], F8, name="wk1_sb")
        for dt in range(DT):
            nc.gpsimd.dma_start(wk1_sb[:, dt * C:(dt + 1) * C],
                                g("wd1m_d")[dt, :, :])
        wd2m_sb = load("wd2m", (128, D), BF16, eng=nc.gpsimd)

        onescb_sb = cst.tile([128, 1], BF16, name="onescb_sb")
        nc.vector.tensor_copy(onescb_sb[:], onesc_sb[:])
        id_f = cst.tile([128, 128], F32, name="id_f")
        masks.make_identity(nc, id_f[:])

        # long-lived results
        qdT_bf = cst.tile([128, HW], BF16, name="qdT_bf")
        kdT_bf = cst.tile([128, HW], BF16, name="kdT_bf")
        kdT_f8 = cst.tile([128, HW], F8, name="kdT_f8")
        qdT_f8 = cst.tile([128, HW], F8, name="qdT_f8")
        qall_sb = cst.tile([128, NT], F8, name="qall_sb")
        qgT_bf = cst.tile([128, 8], BF16, name="qgT_bf")
        kgT_bf = cst.tile([128, 8], BF16, name="kgT_bf")
        qgT_f8 = cst.tile([128, 8], F8, name="qgT_f8")
        pool_sb = cst.tile([128, 16], F32, name="pool_sb")
        gqall = cst.tile([128, 64], F32, name="gqall")  # pooled q [c, (r t)]
        gkall = cst.tile([128, 64], F32, name="gkall")
        cpartb = cst.tile([128, 7], BF16, name="cpartb")  # histogram counts
        fin_sb = cst.tile([1, 16], F32, name="fin_sb")

        ctx = dict(locals())
        _dense(nc, tc, rg, env, cst, ctx)
        _ghead(nc, tc, rg, env, cst, ctx)
        _simhist(nc, tc, rg, env, cst, ctx)
        _logits(nc, tc, rg, env, cst, ctx)


def _dense(nc, tc, rg, env, cst, ctx):
    g = lambda k: env[k]
    c = lambda k: ctx[k]
    pool_in, pool_out = g("pool_in"), g("pool_out")
    qd_in, qd_out = g("qd_in"), g("qd_out")
    xq_sb, xk_sb = c("xq_sb"), c("xk_sb")
    pool_sb = c("pool_sb")

    with tc.tile_pool(name="hp", bufs=3) as hp, \
         tc.tile_pool(name="l2s", bufs=2) as l2s, \
         tc.tile_pool(name="plp", bufs=2) as plp, \
         tc.tile_pool(name="ps_big", bufs=2, space="PSUM") as ps_big, \
         tc.tile_pool(name="ps_qd", bufs=2, space="PSUM") as ps_qd:

        def dense_branch(br, xs, w1sb, w2sb, b1, b2, dst, dst8):
            qd_ps = ps_qd.tile([128, HW], F32, name="qd_ps", tag="qd")
            for dt in range(DT):
                w1t = w1sb[:, dt * C:(dt + 1) * C]
                h_ps = ps_big.tile([128, HW], F32, name="h_ps", tag="big")
                if USE_DR:
                    for cp in range(CT // 2):
                        wp = w1t[:, cp * 256:(cp + 1) * 256].rearrange(
                            "p (two m) -> p two m", two=2)
                        xp = xs[:, cp * 2 * HW:(cp + 1) * 2 * HW].rearrange(
                            "p (two n) -> p two n", two=2)
                        for (o, n) in _chunks(HW):
                            nc.tensor.matmul(
                                h_ps[:, o:o + n],
                                lhsT=wp,
                                rhs=xp[:, :, o:o + n],
                                start=(cp == 0), stop=(cp == CT // 2 - 1),
                                perf_mode=DR)
                else:
                    for ct in range(CT):
                        for (o, n) in _chunks(HW):
                            nc.tensor.matmul(
                                h_ps[:, o:o + n],
                                lhsT=w1t[:, ct * 128:(ct + 1) * 128],
                                rhs=xs[:, ct * HW + o:ct * HW + o + n],
                                start=(ct == 0), stop=(ct == CT - 1))
                h_sb = hp.tile([128, HW], BF16, name="h_sb")
                nc.scalar.activation(h_sb[:], h_ps[:], AF.Relu,
                                     bias=b1[:, dt:dt + 1])
                if br == 0 and dt in (2, 4, 6, 8):
                    # pooling of xq/xk on DVE while PE grinds L1
                    base = 0 if dt in (2, 4) else 8
                    src = xq_sb if dt in (2, 4) else xk_sb
                    c0 = 0 if dt in (2, 6) else 4
                    for ct2 in range(c0, c0 + 4):
                        scr = plp.tile([128, HW], BF16, name="pool_scr")
                        nc.vector.tensor_scalar(
                            scr[:], src[:, ct2 * HW:(ct2 + 1) * HW], 1.0,
                            None, op0=ALU.mult, op1=ALU.add,
                            accum_out=pool_sb[:, base + ct2:base + ct2 + 1])
                for (o, n) in _chunks(HW):
                    nc.tensor.matmul(
                        qd_ps[:, o:o + n],
                        lhsT=w2sb[:, dt * 128:(dt + 1) * 128],
                        rhs=h_sb[:, o:o + n],
                        start=(dt == 0), stop=(dt == DT - 1))
            # bias + l2-normalize along channels (partition dim)
            qdT_f = l2s.tile([128, HW], F32, name="qdT_f")
            nc.scalar.activation(qdT_f[:], qd_ps[:], AF.Identity, bias=b2[:])
            sq = l2s.tile([128, HW], BF16, name="sq")
            nc.scalar.activation(sq[:], qdT_f[:], AF.Square)
            ssq_ps = ps_qd.tile([1, HW], F32, name="ssq_ps", tag="qd")
            for (o, n) in _chunks(HW):
                nc.tensor.matmul(ssq_ps[:, o:o + n], lhsT=c("onescb_sb")[:],
                                 rhs=sq[:, o:o + n], start=True, stop=True)
            nrm = l2s.tile([1, HW], F32, name="nrm")
            nc.vector.tensor_scalar_max(nrm[:], ssq_ps[:], 1e-12)
            # rsqrt(s) = exp(-0.5*ln(s)) keeps ACT on one table set
            nrm2 = l2s.tile([1, HW], F32, name="nrm2")
            nc.scalar.activation(nrm2[:], nrm[:], AF.Ln)
            rn = l2s.tile([1, HW], F32, name="rn")
            nc.scalar.activation(rn[:], nrm2[:], AF.Exp, scale=-0.5)
            rnb_ps = ps_qd.tile([128, HW], F32, name="rnb_ps", tag="qd")
            for (o, n) in _chunks(HW):
                nc.tensor.matmul(rnb_ps[:, o:o + n], lhsT=c("onesr_sb")[:],
                                 rhs=rn[:, o:o + n], start=True, stop=True)
            nc.vector.tensor_mul(dst[:], qdT_f[:], rnb_ps[:])
            nc.vector.tensor_copy(dst8[:], dst[:])

        dense_branch(0, xq_sb, c("wq1_sb"), c("wd2_sb"), c("bd1_sb"),
                     c("bd2_sb"), c("qdT_bf"), c("qdT_f8"))
        # ship q_d: this collective gates the whole logits tail, so it
        # goes first on the collective stream
        nc.sync.dma_start(
            qd_in[:].rearrange("(c p) -> c p", c=128),
            c("qdT_f8")[:].bitcast(F32))
        nc.gpsimd.collective_compute(
            "AllGather", ALU.bypass, replica_groups=rg,
            ins=[qd_in.opt()], outs=[qd_out.opt()])
        # tail inputs ride the gpsimd ring while it is blocked on the AG
        queueT_sb = cst.tile([128, QSH], F8, name="queueT_sb")
        nc.gpsimd.dma_start(queueT_sb[:], g("queueT_d")[:])
        ctx["queueT_sb"] = queueT_sb
        for nm in ("bg1", "bg1m", "bg2", "bg2m"):
            t = cst.tile([128, GDT] if nm in ("bg1", "bg1m") else [128, 1],
                         F32, name=nm + "_sb")
            nc.gpsimd.dma_start(t[:], g(nm + "_d")[:])
            ctx[nm + "_sb"] = t
        for nm, w in (("wg1", CT * GDT * 128), ("wg1m", CT * GDT * 128),
                      ("wg2", GDT * 128), ("wg2m", GDT * 128)):
            t = cst.tile([128, w], BF16, name=nm + "_sb")
            nc.gpsimd.dma_start(t[:], g(nm + "_d")[:])
            ctx[nm + "_sb"] = t
        # pooled features AllGather (second on the stream)
        pin = pool_in[:].rearrange("(g t c) -> c (g t)", g=2, t=8, c=128)
        nc.gpsimd.dma_start(pin, pool_sb[:])
        nc.gpsimd.collective_compute(
            "AllGather", ALU.bypass, replica_groups=rg,
            ins=[pool_in.opt()], outs=[pool_out.opt()])
        # qall load split across the sync and tensor rings
        for r in range(8):
            eng = nc.sync if r % 2 == 0 else nc.scalar
            eng.dma_start(
                c("qall_sb")[:, r * HW:(r + 1) * HW].bitcast(F32),
                qd_out[r * 128 * HW // 4:(r + 1) * 128 * HW // 4]
                .rearrange("(c p) -> c p", c=128))
        # pooled features for every image: [c, (r t)] layout
        with tc.tile_wait_until(0.095):
            pg = pool_out[:].rearrange("(r g x) -> r g x", r=8, g=2)
            for gi, dstp in ((0, c("gqall")), (1, c("gkall"))):
                for r in range(8):
                    nc.sync.dma_start(
                        dstp[:, r * 8:(r + 1) * 8],
                        pg[r, gi, :].rearrange("(t c) -> c t", c=128))

        dense_branch(1, xk_sb, c("wk1_sb"), c("wd2m_sb"), c("bd1m_sb"),
                     c("bd2m_sb"), c("kdT_bf"), c("kdT_f8"))


def _ghead(nc, tc, rg, env, cst, ctx):
    """D-sharded global heads (2 of 16 D-tiles per core) + AllReduce."""
    g = lambda k: env[k]
    c = lambda k: ctx[k]

    with tc.tile_wait_until(0.100), \
         tc.tile_pool(name="gh", bufs=1) as gh, \
         tc.tile_pool(name="ps_gh", bufs=2, space="PSUM") as ps_gh, \
         tc.tile_pool(name="ps_gq", bufs=1, space="PSUM") as ps_gq:
        gq_bf = gh.tile([128, 64], BF16, name="gq_bf")
        gk_bf = gh.tile([128, 64], BF16, name="gk_bf")
        nc.vector.tensor_scalar_mul(gq_bf[:], c("gqall")[:], 1.0 / HW)
        nc.vector.tensor_scalar_mul(gk_bf[:], c("gkall")[:], 1.0 / HW)
        gprt = gh.tile([128, 16], F32, name="gprt")
        for br2, (gsb, w1sb, w2sb, b1c) in enumerate([
                (gq_bf, c("wg1_sb"), c("wg2_sb"), c("bg1_sb")),
                (gk_bf, c("wg1m_sb"), c("wg2m_sb"), c("bg1m_sb"))]):
            gv = gsb[:].rearrange("c (r t) -> c t r", t=8)
            qg_ps = ps_gq.tile([128, 8], F32, name="qg_ps", tag="qg")
            for dl in range(GDT):
                hgt_ps = ps_gh.tile([128, 8], F32, name="hgt_ps")
                for ct in range(CT):
                    nc.tensor.matmul(
                        hgt_ps[:],
                        lhsT=w1sb[:, (ct * GDT + dl) * 128:
                                  (ct * GDT + dl + 1) * 128],
                        rhs=gv[:, ct, :],
                        start=(ct == 0), stop=(ct == CT - 1))
                hgt_sb = gh.tile([128, 8], BF16, name=f"hgt{br2}_{dl}")
                nc.vector.tensor_scalar(hgt_sb[:], hgt_ps[:],
                                        b1c[:, dl:dl + 1], 0.0,
                                        op0=ALU.add, op1=ALU.max)
                nc.tensor.matmul(qg_ps[:],
                                 lhsT=w2sb[:, dl * 128:(dl + 1) * 128],
                                 rhs=hgt_sb[:], start=(dl == 0),
                                 stop=(dl == GDT - 1))
            nc.vector.tensor_copy(gprt[:, br2 * 8:br2 * 8 + 8], qg_ps[:])
        nc.gpsimd.dma_start(
            g("gar_in")[:].rearrange("(c p) -> c p", c=128), gprt[:])
        nc.gpsimd.collective_compute(
            "AllReduce", ALU.add, replica_groups=rg,
            ins=[g("gar_in").opt()], outs=[g("gar_out").opt()])


def _simhist(nc, tc, rg, env, cst, ctx):
    """sim + argmax + histogram of matched indices + positives partial."""
    g = lambda k: env[k]
    c = lambda k: ctx[k]
    out_d = g("out_d")
    qdT_bf, kdT_bf = c("qdT_bf"), c("kdT_bf")
    fin_sb = c("fin_sb")

    with tc.tile_pool(name="cor", bufs=1) as cor, \
         tc.tile_pool(name="corS", bufs=2) as corS, \
         tc.tile_pool(name="ps_sim", bufs=2, space="PSUM") as ps_sim, \
         tc.tile_pool(name="ps_ir", bufs=2, space="PSUM") as ps_ir:
        sim_sb = cor.tile([128, 7 * HW], BF16, name="sim_sb")
        mx8 = cor.tile([128, 8], BF16, name="mx8")
        ix8 = cor.tile([128, 8], mybir.dt.uint32, name="ix8")
        ixf = cor.tile([128, 7], F32, name="ixf")
        posv = cor.tile([128, 7], F32, name="posv")
        nc.vector.memset(posv[:], 0.0)
        for i, (po_, pn) in enumerate(PT):
            s_ps = ps_sim.tile([128, HW], F32, name="s_ps", tag="sim")
            for (o, n) in _chunks(HW):
                nc.tensor.matmul(s_ps[0:pn, o:o + n],
                                 lhsT=qdT_bf[:, po_:po_ + pn],
                                 rhs=kdT_bf[:, o:o + n],
                                 start=True, stop=True)
            nc.scalar.activation(sim_sb[0:pn, i * HW:i * HW + HW],
                                 s_ps[0:pn, :], AF.Copy)
            nc.vector.max(mx8[0:pn, :], sim_sb[0:pn, i * HW:i * HW + HW])
            nc.vector.max_index(ix8[0:pn, :], mx8[0:pn, :],
                                sim_sb[0:pn, i * HW:i * HW + HW])
            nc.vector.tensor_copy(ixf[0:pn, i:i + 1], ix8[0:pn, 0:1])
            nc.vector.tensor_copy(posv[0:pn, i:i + 1], mx8[0:pn, 0:1])
        # broadcast idx row to all partitions
        ir_sb = cor.tile([1, HW], F32, name="ir_sb")
        for i, (po_, pn) in enumerate(PT):
            ir_ps = ps_ir.tile([1, 128], F32, name="ir_ps", tag="ir")
            nc.tensor.transpose(ir_ps[0:1, 0:pn], ixf[0:pn, i:i + 1],
                                c("id_f")[0:pn, 0:pn])
            nc.scalar.activation(ir_sb[0:1, po_:po_ + pn],
                                 ir_ps[0:1, 0:pn], AF.Copy)
        ib_ps = ps_sim.tile([128, HW], F32, name="ib_ps", tag="sim")
        for (o, n) in _chunks(HW):
            nc.tensor.matmul(ib_ps[:, o:o + n], lhsT=c("onesr_sb")[:],
                             rhs=ir_sb[:, o:o + n], start=True, stop=True)
        ib_sb = cor.tile([128, HW], F32, name="ib_sb")
        nc.scalar.activation(ib_sb[:], ib_ps[:], AF.Copy)
        # histogram: count idx == m via is_equal + free-axis accumulate
        cpart = cor.tile([128, 7], F32, name="cpart")
        nc.vector.memset(cpart[:], 0.0)
        for i, (po_, pn) in enumerate(PT):
            S = corS.tile([128, HW], BF16, name="S")
            nc.vector.tensor_scalar(
                S[0:pn, :], ib_sb[0:pn, :], c("iotap_sb")[0:pn, i:i + 1],
                None, op0=ALU.is_equal, op1=ALU.add,
                accum_out=cpart[0:pn, i:i + 1])
        nc.vector.tensor_copy(c("cpartb")[:], cpart[:])
        # positives partial: sum(max sim) over own rows
        pos_ps = ps_ir.tile([1, 128], F32, name="pos_ps", tag="ir")
        nc.tensor.matmul(pos_ps[0:1, 0:7], lhsT=c("onesc_sb")[:],
                         rhs=posv[:], start=True, stop=True)
        nc.vector.reduce_sum(fin_sb[0:1, 0:1], pos_ps[0:1, 0:7],
                             axis=mybir.AxisListType.X)
    nc.sync.dma_start(out_d[0:1, NT:NT + 1], fin_sb[0:1, 0:1])


def _logits(nc, tc, rg, env, cst, ctx):
    """Column-sharded dense-InfoNCE partial Z + qg norm/lpos/queue negs.

    [m, i] orientation: P^T tiles (lhsT = k_d m-tile, rhs = q_all), exp to
    E' in SBUF, then Z[i] = sum_m c_m E'[m, i] on the PE with the counts
    column as the stationary operand.
    """
    g = lambda k: env[k]
    c = lambda k: ctx[k]
    out_d = g("out_d")
    gar_out = g("gar_out")
    qall_sb, kdT_f8 = c("qall_sb"), c("kdT_f8")
    fin_sb, cpartb = c("fin_sb"), c("cpartb")
    qgT_bf, kgT_bf, qgT_f8 = c("qgT_bf"), c("kgT_bf"), c("qgT_f8")

    with tc.tile_pool(name="lg", bufs=1) as lgp, \
         tc.tile_pool(name="gn", bufs=1) as gn, \
         tc.tile_pool(name="ps_lg", bufs=2, space="PSUM") as ps_lg, \
         tc.tile_pool(name="ps_z", bufs=2, space="PSUM") as ps_z, \
         tc.tile_pool(name="ps_gn", bufs=1, space="PSUM") as ps_gn:
        E_all = lgp.tile([128, 7 * NT], BF16, name="E_all")
        z_row = lgp.tile([1, NT], F32, name="z_row")
        # P^T + exp, i-chunk-major so the Z sums can chase the exp chain
        for (co, cn) in _chunks(NT, 1024):
            for j, (po_, pn) in enumerate(PT):
                lg_ps = ps_lg.tile([128, 1024], F32, name="lg_ps")
                for (o, n) in _chunks(cn):
                    nc.tensor.matmul(
                        lg_ps[0:pn, o:o + n],
                        lhsT=kdT_f8[:, po_:po_ + pn],
                        rhs=qall_sb[:, co + o:co + o + n],
                        start=True, stop=True)
                nc.scalar.activation(E_all[0:pn, j * NT + co:j * NT + co + cn],
                                     lg_ps[0:pn, 0:cn], AF.Exp, scale=ISC)

        # ---- Z[i] = sum_m c_m E'[m, i] on the PE, chasing the exp chain
        for (co, cn) in _chunks(NT):
            zc_ps = ps_z.tile([1, 512], F32, name="zc_ps", tag="z")
            for j, (po_, pn) in enumerate(PT):
                nc.tensor.matmul(zc_ps[0:1, 0:cn],
                                 lhsT=cpartb[0:pn, j:j + 1],
                                 rhs=E_all[0:pn, j * NT + co:j * NT + co + cn],
                                 start=(j == 0), stop=(j == 6))
            nc.scalar.activation(z_row[0:1, co:co + cn], zc_ps[0:1, 0:cn],
                                 AF.Copy)
        nc.sync.dma_start(out_d[0:1, 0:NT], z_row[:])

        # ---- qg/kg bias + l2 norm + lpos + queue negatives (gated on the
        # ghead AllReduce; scheduled late so they never stall the exp chain)
        with tc.tile_wait_until(0.115):
            garT = gn.tile([128, 16], F32, name="garT")
            nc.gpsimd.dma_start(
                garT[:], gar_out[:].rearrange("(c p) -> c p", c=128))
            for br2, (b2, dstg) in enumerate([(c("bg2_sb"), qgT_bf),
                                              (c("bg2m_sb"), kgT_bf)]):
                qgT_f = gn.tile([128, 8], F32, name=f"qgT_f{br2}")
                nc.scalar.activation(qgT_f[:], garT[:, br2 * 8:br2 * 8 + 8],
                                     AF.Identity, bias=b2[:])
                sqg = gn.tile([128, 8], BF16, name=f"sqg{br2}")
                nc.scalar.activation(sqg[:], qgT_f[:], AF.Square)
                ssg_ps = ps_gn.tile([1, 8], F32, name="ssg_ps", tag="g8")
                nc.tensor.matmul(ssg_ps[:], lhsT=c("onescb_sb")[:],
                                 rhs=sqg[:], start=True, stop=True)
                nrg = gn.tile([1, 8], F32, name=f"nrg{br2}")
                nc.vector.tensor_scalar_max(nrg[:], ssg_ps[:], 1e-12)
                nrg2 = gn.tile([1, 8], F32, name=f"nrg2{br2}")
                nc.scalar.activation(nrg2[:], nrg[:], AF.Ln)
                rng = gn.tile([1, 8], F32, name=f"rng{br2}")
                nc.scalar.activation(rng[:], nrg2[:], AF.Exp, scale=-0.5)
                rngb_ps = ps_gn.tile([128, 8], F32, name="rngb_ps", tag="g8")
                nc.tensor.matmul(rngb_ps[:], lhsT=c("onesr_sb")[:],
                                 rhs=rng[:], start=True, stop=True)
                nc.vector.tensor_mul(dstg[:], qgT_f[:], rngb_ps[:])
            lpm = gn.tile([128, 8], F32, name="lpm")
            nc.vector.tensor_mul(lpm[:], qgT_bf[:], kgT_bf[:])
            lp_ps = ps_gn.tile([1, 8], F32, name="lp_ps", tag="g8")
            nc.tensor.matmul(lp_ps[:], lhsT=c("onesc_sb")[:], rhs=lpm[:],
                             start=True, stop=True)
            nc.scalar.activation(fin_sb[0:1, 1:9], lp_ps[:], AF.Copy)
            nc.sync.dma_start(out_d[0:1, NT + 9:NT + 17], fin_sb[0:1, 1:9])
            nc.vector.tensor_copy(qgT_f8[:], qgT_bf[:])
            # queue negatives: transposed orientation, single exp
            qe_ps = ps_z.tile([128, 512], F32, name="qe_ps", tag="z")
            for qt in range(64):
                nc.tensor.matmul(
                    qe_ps[:, qt * 8:(qt + 1) * 8],
                    lhsT=c("queueT_sb")[:, qt * 128:(qt + 1) * 128],
                    rhs=qgT_f8[:], start=True, stop=True)
            qe_sb = gn.tile([128, 512], BF16, name="qe_sb")
            nc.scalar.activation(qe_sb[:], qe_ps[:], AF.Exp, scale=ISC)
            qs_ps = ps_gn.tile([1, 512], F32, name="qs_ps", tag="g8")
            for (o, n) in _chunks(512):
                nc.tensor.matmul(qs_ps[:, o:o + n], lhsT=c("onescb_sb")[:],
                                 rhs=qe_sb[:, o:o + n], start=True, stop=True)
            qsum_r = gn.tile([1, 8], F32, name="qsum_r")
            nc.vector.reduce_sum(qsum_r[:],
                                 qs_ps[:].rearrange("p (t i) -> p i t", i=8),
                                 axis=mybir.AxisListType.X)
            nc.sync.dma_start(out_d[0:1, NT + 1:NT + 9], qsum_r[:])



def _prep_inputs(inputs):
    fq = np.asarray(inputs["feat_q"], np.float32).reshape(B, HW, C)
    fk = np.asarray(inputs["feat_k"], np.float32).reshape(B, HW, C)

    def xT(x):  # (784, 1024) -> (128, 8*784) fp8 with [c, ct*784+p]
        return np.ascontiguousarray(
            x.reshape(HW, CT, 128).transpose(2, 1, 0).reshape(128, CT * HW)
        ).astype(F8NP)

    def w1tile(w):  # (1024, 2048) -> (16, 128, 1024) fp8, scaled
        return np.ascontiguousarray(
            (w * WSCALE).reshape(CT, 128, DT, 128).transpose(2, 1, 0, 3)
            .reshape(DT, 128, C)).astype(F8NP)

    def w2tile(w):  # (2048, 128) -> (128, 2048) with [c, dt*128+d]
        return np.ascontiguousarray(
            w.reshape(DT, 128, 128).transpose(1, 0, 2).reshape(128, D)
        ).astype(BF)

    queue = np.asarray(inputs["queue"], np.float32)
    wg1 = np.asarray(inputs["Wg1"], np.float32)   # (1024, 2048)
    wg1m = np.asarray(inputs["mWg1"], np.float32)
    wg2 = np.asarray(inputs["Wg2"], np.float32)   # (2048, 128)
    wg2m = np.asarray(inputs["mWg2"], np.float32)
    bg1 = np.asarray(inputs["bg1"], np.float32)
    bg1m = np.asarray(inputs["mbg1"], np.float32)

    iotap = (np.arange(128, dtype=np.float32)[:, None]
             + 128.0 * np.arange(8, dtype=np.float32)[None, :])

    shared = {
        "wd1": w1tile(np.asarray(inputs["Wd1"], np.float32)),
        "wd1m": w1tile(np.asarray(inputs["mWd1"], np.float32)),
        "wd2": w2tile(np.asarray(inputs["Wd2"], np.float32)),
        "wd2m": w2tile(np.asarray(inputs["mWd2"], np.float32)),
        "bd1": np.ascontiguousarray(
            (np.asarray(inputs["bd1"], np.float32) * WSCALE)
            .reshape(DT, 128).T).astype(np.float32),
        "bd1m": np.ascontiguousarray(
            (np.asarray(inputs["mbd1"], np.float32) * WSCALE)
            .reshape(DT, 128).T).astype(np.float32),
        "bd2": (np.asarray(inputs["bd2"], np.float32) * WSCALE
                ).reshape(128, 1),
        "bd2m": (np.asarray(inputs["mbd2"], np.float32) * WSCALE
                 ).reshape(128, 1),
        "bg2": np.asarray(inputs["bg2"], np.float32).reshape(128, 1),
        "bg2m": np.asarray(inputs["mbg2"], np.float32).reshape(128, 1),
        "iotap": np.ascontiguousarray(iotap),
        "onesc": np.ones((128, 1), np.float32),
        "onesr": np.ones((1, 128), np.float32),
    }
    in_maps = []
    for cc in range(N_CORES):
        m = dict(shared)
        m["xq"] = xT(fq[cc])
        m["xk"] = xT(fk[cc])
        m["queueT"] = np.ascontiguousarray(
            queue[cc * QSH:(cc + 1) * QSH].T).astype(F8NP)
        # per-core D-slice of the global head: dts {2c, 2c+1}
        dsl = slice(cc * GDT * 128, (cc + 1) * GDT * 128)
        # wg1 slice layout [c, (ct*GDT+dl)*128 + d]
        m["wg1"] = np.ascontiguousarray(
            wg1[:, dsl].reshape(CT, 128, GDT * 128).transpose(1, 0, 2)
            .reshape(128, CT * GDT * 128)).astype(BF)
        m["wg1m"] = np.ascontiguousarray(
            wg1m[:, dsl].reshape(CT, 128, GDT * 128).transpose(1, 0, 2)
            .reshape(128, CT * GDT * 128)).astype(BF)
        # wg2 slice [dl*128+r, P] -> lhsT layout [r, dl*128+p]
        m["wg2"] = np.ascontiguousarray(
            wg2[dsl].reshape(GDT, 128, 128).transpose(1, 0, 2)
            .reshape(128, GDT * 128)).astype(BF)
        m["wg2m"] = np.ascontiguousarray(
            wg2m[dsl].reshape(GDT, 128, 128).transpose(1, 0, 2)
            .reshape(128, GDT * 128)).astype(BF)
        m["bg1"] = np.ascontiguousarray(
            bg1[dsl].reshape(GDT, 128).T).astype(np.float32)
        m["bg1m"] = np.ascontiguousarray(
            bg1m[dsl].reshape(GDT, 128).T).astype(np.float32)
        in_maps.append(m)
    return in_maps


_NC = None


def _get_nc():
    global _NC
    if _NC is None:
        _NC = _build()
    return _NC


def _host_combine(outs):
    """outs: [8, 1, OUTW] per-core partial rows -> final scalar loss.

    Per core: [0:6272] Z row-sum partials over its 784 logit columns,
    [6272] partial sum(max sim) over its rows, [6273:6281] partial
    sum(exp(l_neg/tau)) per image over its queue shard, [6281:6289]
    l_pos per image (replicated).
    """
    outs = np.asarray(outs, np.float64).reshape(len(outs), -1)
    Zf = outs[:, 0:NT].sum(axis=0)
    possum = outs[:, NT].sum()
    l_d = np.mean(np.log(Zf)) - ISC * possum / NT
    qsums = outs[:, NT + 1:NT + 9].sum(axis=0)
    lpos = outs[0, NT + 9:NT + 17]
    lse = np.log(np.exp(ISC * lpos) + qsums)
    l_g = np.mean(lse - ISC * lpos)
    return np.float32((1.0 - LAM) * l_g + LAM * l_d).reshape(())


def kernel(**inputs) -> np.ndarray:
    nc = _get_nc()
    in_maps = _prep_inputs(inputs)
    res = bass_utils.run_bass_kernel_spmd(nc, in_maps,
                                          core_ids=list(range(N_CORES)))
    outs = np.stack([res.results[c]["out"].reshape(1, OUTW)
                     for c in range(N_CORES)])
    return _host_combine(outs)


# revision 29
# speedup vs baseline: 1.0810x; 1.0292x over previous
"""DenseCL loss kernel for 8 TRN2 NeuronCores.

Sharding: core c owns batch image c for the dense branch, queue rows
[c*8192, (c+1)*8192) for the queue-InfoNCE negatives, and the COLUMN block
[c*784, (c+1)*784) of the flat dense-InfoNCE logits.

Key identity: matched_k[j] = k_d[:, idx_j], so the dense logits matrix is a
column gather of P = k_d_local^T @ q_all.  Each core computes partial row
sums Z_i = sum_m c_m * exp(P[m, i] / tau) where c is the histogram of its
own argmax indices (the weighted partition sum runs on the PE with the
counts as a stationary column), and the positives are the sim row maxima.
No matched-key gather and no matched-key AllGather is needed.

Collectives (gpsimd stream): a dummy 32-byte AllGather issued first thing
absorbs the cross-core start-skew barrier; then AllGather of pooled
features, AllGather of normalized q_d (fp8 bytes moved as f32 elements,
hidden under the k branch), AllReduce of the D-sharded global-head
partials.  Final ~10K-flop unshard happens on the host.
"""
import os
import sys

if "/opt/trn_rl_repo" not in sys.path:
    sys.path.insert(0, "/opt/trn_rl_repo")

USE_DR = os.environ.get("KDR", "1") == "1"      # fp8 DoubleRow for dense L1

import numpy as np
import ml_dtypes

import concourse.bass as bass
import concourse.bacc as bacc
import concourse.mybir as mybir
import concourse.tile as tile
from concourse import bass_utils, masks

BF = ml_dtypes.bfloat16
F8NP = ml_dtypes.float8_e4m3
F32 = mybir.dt.float32
BF16 = mybir.dt.bfloat16
F8 = mybir.dt.float8e4
DR = mybir.MatmulPerfMode.DoubleRow

N_CORES = 8
B, HW, C, D, P, Q = 8, 784, 1024, 2048, 128, 65536
QSH = Q // N_CORES          # 8192 queue rows per core
CT, DT = C // 128, D // 128  # 8, 16
GDT = DT // N_CORES         # 2 ghead D-tiles per core
NT = B * HW                 # 6272 total dense rows
TAU = 0.2
LAM = 0.5
ISC = 1.0 / TAU             # 5.0
WSCALE = 32.0               # fp8 range scale for W1/b1 (cancelled by l2 norm)
AF = mybir.ActivationFunctionType
ALU = mybir.AluOpType

# 784 = 6*128 + 16 partition tiles
PT = [(i * 128, min(128, HW - i * 128)) for i in range(7)]
OUTW = 8192                 # out row: [0:6272] Z, 6272 possum,
                            # [6273:6281] qsums, [6281:6289] lpos


def _chunks(n, step=512):
    return [(o, min(step, n - o)) for o in range(0, n, step)]


def _patch_act_tables():
    """Force every activation we use onto the natural_log_exp_and_others
    table set so the kernel needs exactly one ACT_TABLE_LOAD."""
    from concourse import hw_specs
    import concourse.bacc as bacc_mod
    if getattr(bacc_mod, "_act_tables_patched", False):
        return
    orig = hw_specs.get_activation_tables
    ours = {AF.Exp, AF.Ln, AF.Relu, AF.Identity, AF.Copy, AF.Square}
    keep = "natural_log_exp_and_others"

    def patched(arch):
        tabs = orig(arch)
        assert keep in tabs and ours <= tabs[keep]
        return {name: (fns if name == keep else fns - ours)
                for name, fns in tabs.items()}

    bacc_mod.get_activation_tables = patched
    bacc_mod._act_tables_patched = True


def _build(do_compile=True):
    _patch_act_tables()
    nc = bacc.Bacc("TRN2", target_bir_lowering=False, debug=False,
                   num_devices=N_CORES)

    def inp(name, shape, dt):
        return nc.dram_tensor(name, list(shape), dt, kind="ExternalInput")

    xq_d = inp("xq", (128, CT * HW), F8)          # [c, ct*784+p] = feat_q[b, p, ct*128+c]
    xk_d = inp("xk", (128, CT * HW), F8)
    wd1_d = inp("wd1", (DT, 128, C), F8)          # [dt, c, ct*128+d] = 32*Wd1[ct*128+c, dt*128+d]
    wd1m_d = inp("wd1m", (DT, 128, C), F8)
    wd2_d = inp("wd2", (128, D), BF16)            # [c, dt*128+d] = Wd2[dt*128+c, d]
    wd2m_d = inp("wd2m", (128, D), BF16)
    wg1_d = inp("wg1", (128, CT * GDT * 128), BF16)  # per-core D-slice of Wg1
    wg1m_d = inp("wg1m", (128, CT * GDT * 128), BF16)
    wg2_d = inp("wg2", (128, GDT * 128), BF16)    # per-core D-slice of Wg2 (lhsT)
    wg2m_d = inp("wg2m", (128, GDT * 128), BF16)
    bd1_d = inp("bd1", (128, DT), F32)            # [r, dt] = 32*bd1[dt*128+r]
    bd1m_d = inp("bd1m", (128, DT), F32)
    bd2_d = inp("bd2", (128, 1), F32)             # 32*bd2
    bd2m_d = inp("bd2m", (128, 1), F32)
    bg1_d = inp("bg1", (128, GDT), F32)           # per-core D-slice of bg1
    bg1m_d = inp("bg1m", (128, GDT), F32)
    bg2_d = inp("bg2", (128, 1), F32)
    bg2m_d = inp("bg2m", (128, 1), F32)
    queueT_d = inp("queueT", (128, QSH), F8)      # [ch, j] = queue[c0+j, ch]
    iotap_d = inp("iotap", (128, 8), F32)         # col i = p + 128*i
    onesc_d = inp("onesc", (128, 1), F32)         # ones column (lhsT partition sums)
    onesr_d = inp("onesr", (1, 128), F32)         # ones row (lhsT for K=1 broadcast)

    out_d = nc.dram_tensor("out", [1, OUTW], F32, kind="ExternalOutput")

    with tile.TileContext(nc) as tc:
        rg = [list(range(N_CORES))]
        with tc.tile_pool(name="dramp", bufs=1, space="DRAM") as dpool:
            pool_in = dpool.tile([2 * C], F32, name="pool_in")
            pool_out = dpool.tile([N_CORES * 2 * C], F32, name="pool_out",
                                  addr_space="Shared")
            # q_d fp8 bytes shipped as f32 elements (4x fewer CCE elements)
            qd_in = dpool.tile([128 * HW // 4], F32, name="qd_in")
            qd_out = dpool.tile([N_CORES * 128 * HW // 4], F32, name="qd_out",
                                addr_space="Shared")
            gar_in = dpool.tile([128 * 16], F32, name="gar_in")
            gar_out = dpool.tile([128 * 16], F32, name="gar_out",
                                 addr_space="Shared")
            _body(nc, tc, rg, locals())
    if do_compile:
        nc.compile()
    return nc


def _body(nc, tc, rg, env):
    g = lambda k: env[k]

    with tc.tile_pool(name="cst", bufs=1) as cst:

        def load(name, shape, dt, eng=None):
            t = cst.tile(list(shape), dt, name=name + "_sb")
            (eng or nc.sync).dma_start(t[:], g(name + "_d")[:])
            return t

        iotap_sb = load("iotap", (128, 8), F32, eng=nc.gpsimd)
        onesc_sb = load("onesc", (128, 1), F32, eng=nc.gpsimd)
        onesr_sb = load("onesr", (1, 128), F32, eng=nc.gpsimd)

        # ---- sync ring: q-branch critical inputs (per-dt weight slices)
        bd1_sb = load("bd1", (128, DT), F32)
        bd2_sb = load("bd2", (128, 1), F32)
        xq_sb = cst.tile([128, CT * HW], F8, name="xq_sb")
        nc.sync.dma_start(xq_sb[:, 0:4 * HW], g("xq_d")[:, 0:4 * HW])
        wq1_sb = cst.tile([128, DT * C], F8, name="wq1_sb")
        wd2_sb = cst.tile([128, D], BF16, name="wd2_sb")
        for dt in range(DT):
            nc.sync.dma_start(wq1_sb[:, dt * C:(dt + 1) * C],
                              g("wd1_d")[dt, :, :])
            if dt == 0:
                nc.sync.dma_start(xq_sb[:, 4 * HW:CT * HW],
                                  g("xq_d")[:, 4 * HW:CT * HW])
            if dt == 2:
                nc.sync.dma_start(wd2_sb[:], g("wd2_d")[:])
        bd1m_sb = load("bd1m", (128, DT), F32)
        bd2m_sb = load("bd2m", (128, 1), F32)

        # ---- gpsimd (SWDGE) ring: k-branch + tail inputs, so the scalar
        # queue carries only ACT work (DMA triggers head-of-line-block an
        # engine queue once the ring fills)
        xk_sb = cst.tile([128, CT * HW], F8, name="xk_sb")
        nc.gpsimd.dma_start(xk_sb[:], g("xk_d")[:])
        wk1_sb = cst.tile([128, DT * C], F8, name="wk1_sb")
        for dt in range(DT):
            nc.gpsimd.dma_start(wk1_sb[:, dt * C:(dt + 1) * C],
                                g("wd1m_d")[dt, :, :])
        wd2m_sb = load("wd2m", (128, D), BF16, eng=nc.gpsimd)

        onescb_sb = cst.tile([128, 1], BF16, name="onescb_sb")
        nc.vector.tensor_copy(onescb_sb[:], onesc_sb[:])
        id_f = cst.tile([128, 128], F32, name="id_f")
        masks.make_identity(nc, id_f[:])

        # long-lived results
        qdT_bf = cst.tile([128, HW], BF16, name="qdT_bf")
        kdT_bf = cst.tile([128, HW], BF16, name="kdT_bf")
        kdT_f8 = cst.tile([128, HW], F8, name="kdT_f8")
        qdT_f8 = cst.tile([128, HW], F8, name="qdT_f8")
        qall_sb = cst.tile([128, NT], F8, name="qall_sb")
        qgT_bf = cst.tile([128, 8], BF16, name="qgT_bf")
        kgT_bf = cst.tile([128, 8], BF16, name="kgT_bf")
        qgT_f8 = cst.tile([128, 8], F8, name="qgT_f8")
        pool_sb = cst.tile([128, 16], F32, name="pool_sb")
        gqall = cst.tile([128, 64], F32, name="gqall")  # pooled q [c, (r t)]
        gkall = cst.tile([128, 64], F32, name="gkall")
        fin_sb = cst.tile([1, 16], F32, name="fin_sb")

        ctx = dict(locals())
        _dense(nc, tc, rg, env, cst, ctx)
        _tail(nc, tc, rg, env, cst, ctx)


def _dense(nc, tc, rg, env, cst, ctx):
    g = lambda k: env[k]
    c = lambda k: ctx[k]
    pool_in, pool_out = g("pool_in"), g("pool_out")
    qd_in, qd_out = g("qd_in"), g("qd_out")
    xq_sb, xk_sb = c("xq_sb"), c("xk_sb")
    pool_sb = c("pool_sb")

    with tc.tile_pool(name="hp", bufs=3) as hp, \
         tc.tile_pool(name="l2s", bufs=2) as l2s, \
         tc.tile_pool(name="plp", bufs=2) as plp, \
         tc.tile_pool(name="ps_big", bufs=2, space="PSUM") as ps_big, \
         tc.tile_pool(name="ps_qd", bufs=2, space="PSUM") as ps_qd:

        def dense_branch(br, xs, w1sb, w2sb, b1, b2, dst, dst8):
            qd_ps = ps_qd.tile([128, HW], F32, name="qd_ps", tag="qd")
            for dt in range(DT):
                w1t = w1sb[:, dt * C:(dt + 1) * C]
                h_ps = ps_big.tile([128, HW], F32, name="h_ps", tag="big")
                if USE_DR:
                    for cp in range(CT // 2):
                        wp = w1t[:, cp * 256:(cp + 1) * 256].rearrange(
                            "p (two m) -> p two m", two=2)
                        xp = xs[:, cp * 2 * HW:(cp + 1) * 2 * HW].rearrange(
                            "p (two n) -> p two n", two=2)
                        for (o, n) in _chunks(HW):
                            nc.tensor.matmul(
                                h_ps[:, o:o + n],
                                lhsT=wp,
                                rhs=xp[:, :, o:o + n],
                                start=(cp == 0), stop=(cp == CT // 2 - 1),
                                perf_mode=DR)
                else:
                    for ct in range(CT):
                        for (o, n) in _chunks(HW):
                            nc.tensor.matmul(
                                h_ps[:, o:o + n],
                                lhsT=w1t[:, ct * 128:(ct + 1) * 128],
                                rhs=xs[:, ct * HW + o:ct * HW + o + n],
                                start=(ct == 0), stop=(ct == CT - 1))
                h_sb = hp.tile([128, HW], BF16, name="h_sb")
                nc.scalar.activation(h_sb[:], h_ps[:], AF.Relu,
                                     bias=b1[:, dt:dt + 1])
                if br == 0 and dt in (2, 4, 6, 8):
                    # pooling of xq/xk on DVE while PE grinds L1
                    base = 0 if dt in (2, 4) else 8
                    src = xq_sb if dt in (2, 4) else xk_sb
                    c0 = 0 if dt in (2, 6) else 4
                    for ct2 in range(c0, c0 + 4):
                        scr = plp.tile([128, HW], BF16, name="pool_scr")
                        nc.vector.tensor_scalar(
                            scr[:], src[:, ct2 * HW:(ct2 + 1) * HW], 1.0,
                            None, op0=ALU.mult, op1=ALU.add,
                            accum_out=pool_sb[:, base + ct2:base + ct2 + 1])
                # L2 for the PREVIOUS dt: its relu finished during this
                # dt's L1 matmuls, so the PE never waits on the ACT chain
                if dt > 0:
                    for (o, n) in _chunks(HW):
                        nc.tensor.matmul(
                            qd_ps[:, o:o + n],
                            lhsT=w2sb[:, (dt - 1) * 128:dt * 128],
                            rhs=h_prev[:, o:o + n],
                            start=(dt == 1), stop=False)
                h_prev = h_sb
            for (o, n) in _chunks(HW):
                nc.tensor.matmul(
                    qd_ps[:, o:o + n],
                    lhsT=w2sb[:, (DT - 1) * 128:DT * 128],
                    rhs=h_prev[:, o:o + n],
                    start=False, stop=True)
            # bias + l2-normalize along channels (partition dim)
            qdT_f = l2s.tile([128, HW], F32, name="qdT_f")
            nc.scalar.activation(qdT_f[:], qd_ps[:], AF.Identity, bias=b2[:])
            sq = l2s.tile([128, HW], BF16, name="sq")
            nc.scalar.activation(sq[:], qdT_f[:], AF.Square)
            ssq_ps = ps_qd.tile([1, HW], F32, name="ssq_ps", tag="qd")
            for (o, n) in _chunks(HW):
                nc.tensor.matmul(ssq_ps[:, o:o + n], lhsT=c("onescb_sb")[:],
                                 rhs=sq[:, o:o + n], start=True, stop=True)
            nrm = l2s.tile([1, HW], F32, name="nrm")
            nc.vector.tensor_scalar_max(nrm[:], ssq_ps[:], 1e-12)
            # rsqrt(s) = exp(-0.5*ln(s)) keeps ACT on one table set
            nrm2 = l2s.tile([1, HW], F32, name="nrm2")
            nc.scalar.activation(nrm2[:], nrm[:], AF.Ln)
            rn = l2s.tile([1, HW], F32, name="rn")
            nc.scalar.activation(rn[:], nrm2[:], AF.Exp, scale=-0.5)
            rnb_ps = ps_qd.tile([128, HW], F32, name="rnb_ps", tag="qd")
            for (o, n) in _chunks(HW):
                nc.tensor.matmul(rnb_ps[:, o:o + n], lhsT=c("onesr_sb")[:],
                                 rhs=rn[:, o:o + n], start=True, stop=True)
            nc.vector.tensor_mul(dst[:], qdT_f[:], rnb_ps[:])
            nc.vector.tensor_copy(dst8[:], dst[:])

        dense_branch(0, xq_sb, c("wq1_sb"), c("wd2_sb"), c("bd1_sb"),
                     c("bd2_sb"), c("qdT_bf"), c("qdT_f8"))
        # ship q_d: this collective gates the whole logits tail, so it
        # goes first on the collective stream
        nc.sync.dma_start(
            qd_in[:].rearrange("(c p) -> c p", c=128),
            c("qdT_f8")[:].bitcast(F32))
        nc.gpsimd.collective_compute(
            "AllGather", ALU.bypass, replica_groups=rg,
            ins=[qd_in.opt()], outs=[qd_out.opt()])
        # tail inputs ride the gpsimd ring while it is blocked on the AG
        queueT_sb = cst.tile([128, QSH], F8, name="queueT_sb")
        nc.gpsimd.dma_start(queueT_sb[:], g("queueT_d")[:])
        ctx["queueT_sb"] = queueT_sb
        for nm in ("bg1", "bg1m", "bg2", "bg2m"):
            t = cst.tile([128, GDT] if nm in ("bg1", "bg1m") else [128, 1],
                         F32, name=nm + "_sb")
            nc.gpsimd.dma_start(t[:], g(nm + "_d")[:])
            ctx[nm + "_sb"] = t
        for nm, w in (("wg1", CT * GDT * 128), ("wg1m", CT * GDT * 128),
                      ("wg2", GDT * 128), ("wg2m", GDT * 128)):
            t = cst.tile([128, w], BF16, name=nm + "_sb")
            nc.gpsimd.dma_start(t[:], g(nm + "_d")[:])
            ctx[nm + "_sb"] = t
        # pooled features AllGather (second on the stream)
        pin = pool_in[:].rearrange("(g t c) -> c (g t)", g=2, t=8, c=128)
        nc.gpsimd.dma_start(pin, pool_sb[:])
        nc.gpsimd.collective_compute(
            "AllGather", ALU.bypass, replica_groups=rg,
            ins=[pool_in.opt()], outs=[pool_out.opt()])
        # qall load split across the sync and tensor rings
        for r in range(8):
            nc.sync.dma_start(
                c("qall_sb")[:, r * HW:(r + 1) * HW].bitcast(F32),
                qd_out[r * 128 * HW // 4:(r + 1) * 128 * HW // 4]
                .rearrange("(c p) -> c p", c=128))
        # pooled features for every image: [c, (r t)] layout
        with tc.tile_wait_until(0.095):
            pg = pool_out[:].rearrange("(r g x) -> r g x", r=8, g=2)
            for gi, dstp in ((0, c("gqall")), (1, c("gkall"))):
                for r in range(8):
                    nc.sync.dma_start(
                        dstp[:, r * 8:(r + 1) * 8],
                        pg[r, gi, :].rearrange("(t c) -> c t", c=128))

        dense_branch(1, xk_sb, c("wk1_sb"), c("wd2m_sb"), c("bd1m_sb"),
                     c("bd2m_sb"), c("kdT_bf"), c("kdT_f8"))


def _tail(nc, tc, rg, env, cst, ctx):
    """Everything after the dense branches, emission-ordered so no engine
    queue ever head-of-line-blocks on late data:

      sim -> argmax(DVE, concurrent) -> P/exp chunks 0-3 -> histogram ->
      ghead partials + AllReduce -> P/exp chunks 4-7 -> Z (PE, chasing) ->
      qg norm + lpos + queue negatives.
    """
    g = lambda k: env[k]
    c = lambda k: ctx[k]
    out_d = g("out_d")
    gar_in, gar_out = g("gar_in"), g("gar_out")
    qdT_bf, kdT_bf = c("qdT_bf"), c("kdT_bf")
    qall_sb, kdT_f8 = c("qall_sb"), c("kdT_f8")
    fin_sb = c("fin_sb")
    qgT_bf, kgT_bf, qgT_f8 = c("qgT_bf"), c("kgT_bf"), c("qgT_f8")

    with tc.tile_pool(name="cor", bufs=1) as cor, \
         tc.tile_pool(name="corS", bufs=2) as corS, \
         tc.tile_pool(name="lg", bufs=1) as lgp, \
         tc.tile_pool(name="gh", bufs=1) as gh, \
         tc.tile_pool(name="ps_lg", bufs=2, space="PSUM") as ps_lg, \
         tc.tile_pool(name="ps_sim", bufs=1, space="PSUM") as ps_sim, \
         tc.tile_pool(name="ps_ir", bufs=1, space="PSUM") as ps_ir, \
         tc.tile_pool(name="ps_z", bufs=1, space="PSUM") as ps_z:

        # ---- 1. sim + argmax (argmax chain runs on DVE concurrent with
        # the P/exp chunks below)
        sim_sb = cor.tile([128, 7 * HW], BF16, name="sim_sb")
        mx8 = cor.tile([128, 8], BF16, name="mx8")
        ix8 = cor.tile([128, 8], mybir.dt.uint32, name="ix8")
        ixf = cor.tile([128, 7], F32, name="ixf")
        posv = cor.tile([128, 7], F32, name="posv")
        nc.vector.memset(posv[:], 0.0)
        for i, (po_, pn) in enumerate(PT):
            s_ps = ps_sim.tile([128, HW], F32, name="s_ps", tag="sim")
            for (o, n) in _chunks(HW):
                nc.tensor.matmul(s_ps[0:pn, o:o + n],
                                 lhsT=qdT_bf[:, po_:po_ + pn],
                                 rhs=kdT_bf[:, o:o + n],
                                 start=True, stop=True)
            nc.scalar.activation(sim_sb[0:pn, i * HW:i * HW + HW],
                                 s_ps[0:pn, :], AF.Copy)
            nc.vector.max(mx8[0:pn, :], sim_sb[0:pn, i * HW:i * HW + HW])
            nc.vector.max_index(ix8[0:pn, :], mx8[0:pn, :],
                                sim_sb[0:pn, i * HW:i * HW + HW])
            nc.vector.tensor_copy(ixf[0:pn, i:i + 1], ix8[0:pn, 0:1])
            nc.vector.tensor_copy(posv[0:pn, i:i + 1], mx8[0:pn, 0:1])

        # ---- 2. P/exp, per-rank chunks (column-sharded logits)
        E_all = lgp.tile([128, 7 * NT], BF16, name="E_all")
        cpartb = cor.tile([128, 7], BF16, name="cpartb")

        def pexp(r):
            co = r * HW
            for j, (po_, pn) in enumerate(PT):
                lg_ps = ps_lg.tile([128, HW], F32, name="lg_ps")
                for (o, n) in _chunks(HW):
                    nc.tensor.matmul(
                        lg_ps[0:pn, o:o + n],
                        lhsT=kdT_f8[:, po_:po_ + pn],
                        rhs=qall_sb[:, co + o:co + o + n],
                        start=True, stop=True)
                nc.scalar.activation(E_all[0:pn, j * NT + co:j * NT + co + HW],
                                     lg_ps[0:pn, :], AF.Exp, scale=ISC)

        for r in range(4):
            pexp(r)

        # ---- 3. histogram of argmax indices + positives partial
        ir_sb = cor.tile([1, HW], F32, name="ir_sb")
        for i, (po_, pn) in enumerate(PT):
            ir_ps = ps_ir.tile([1, 128], F32, name="ir_ps", tag="ir")
            nc.tensor.transpose(ir_ps[0:1, 0:pn], ixf[0:pn, i:i + 1],
                                c("id_f")[0:pn, 0:pn])
            nc.scalar.activation(ir_sb[0:1, po_:po_ + pn],
                                 ir_ps[0:1, 0:pn], AF.Copy)
        ib_ps = ps_sim.tile([128, HW], F32, name="ib_ps", tag="sim")
        for (o, n) in _chunks(HW):
            nc.tensor.matmul(ib_ps[:, o:o + n], lhsT=c("onesr_sb")[:],
                             rhs=ir_sb[:, o:o + n], start=True, stop=True)
        ib_sb = cor.tile([128, HW], F32, name="ib_sb")
        nc.scalar.activation(ib_sb[:], ib_ps[:], AF.Copy)
        cpart = cor.tile([128, 7], F32, name="cpart")
        nc.vector.memset(cpart[:], 0.0)
        for i, (po_, pn) in enumerate(PT):
            S = corS.tile([128, HW], BF16, name="S")
            nc.vector.tensor_scalar(
                S[0:pn, :], ib_sb[0:pn, :], c("iotap_sb")[0:pn, i:i + 1],
                None, op0=ALU.is_equal, op1=ALU.add,
                accum_out=cpart[0:pn, i:i + 1])
        nc.vector.tensor_copy(cpartb[:], cpart[:])
        pos_ps = ps_ir.tile([1, 128], F32, name="pos_ps", tag="ir")
        nc.tensor.matmul(pos_ps[0:1, 0:7], lhsT=c("onesc_sb")[:],
                         rhs=posv[:], start=True, stop=True)
        nc.vector.reduce_sum(fin_sb[0:1, 0:1], pos_ps[0:1, 0:7],
                             axis=mybir.AxisListType.X)
        nc.sync.dma_start(out_d[0:1, NT:NT + 1], fin_sb[0:1, 0:1])

        # ---- 4. global-head partials (D-sharded) + AllReduce
        gq_bf = gh.tile([128, 64], BF16, name="gq_bf")
        gk_bf = gh.tile([128, 64], BF16, name="gk_bf")
        nc.vector.tensor_scalar_mul(gq_bf[:], c("gqall")[:], 1.0 / HW)
        nc.vector.tensor_scalar_mul(gk_bf[:], c("gkall")[:], 1.0 / HW)
        gprt = gh.tile([128, 16], F32, name="gprt")
        for br2, (gsb, w1sb, w2sb, b1c) in enumerate([
                (gq_bf, c("wg1_sb"), c("wg2_sb"), c("bg1_sb")),
                (gk_bf, c("wg1m_sb"), c("wg2m_sb"), c("bg1m_sb"))]):
            gv = gsb[:].rearrange("c (r t) -> c t r", t=8)
            qg_ps = ps_z.tile([128, 8], F32, name="qg_ps", tag="z")
            for dl in range(GDT):
                hgt_ps = ps_ir.tile([128, 8], F32, name="hgt_ps", tag="ir")
                for ct in range(CT):
                    nc.tensor.matmul(
                        hgt_ps[:],
                        lhsT=w1sb[:, (ct * GDT + dl) * 128:
                                  (ct * GDT + dl + 1) * 128],
                        rhs=gv[:, ct, :],
                        start=(ct == 0), stop=(ct == CT - 1))
                hgt_sb = gh.tile([128, 8], BF16, name=f"hgt{br2}_{dl}")
                nc.vector.tensor_scalar(hgt_sb[:], hgt_ps[:],
                                        b1c[:, dl:dl + 1], 0.0,
                                        op0=ALU.add, op1=ALU.max)
                nc.tensor.matmul(qg_ps[:],
                                 lhsT=w2sb[:, dl * 128:(dl + 1) * 128],
                                 rhs=hgt_sb[:], start=(dl == 0),
                                 stop=(dl == GDT - 1))
            nc.vector.tensor_copy(gprt[:, br2 * 8:br2 * 8 + 8], qg_ps[:])
        nc.gpsimd.dma_start(
            gar_in[:].rearrange("(c p) -> c p", c=128), gprt[:])
        nc.gpsimd.collective_compute(
            "AllReduce", ALU.add, replica_groups=rg,
            ins=[gar_in.opt()], outs=[gar_out.opt()])
        garT = gh.tile([128, 16], F32, name="garT")
        nc.gpsimd.dma_start(
            garT[:], gar_out[:].rearrange("(c p) -> c p", c=128))

        # ---- 5. remaining P/exp chunks
        for r in range(4, 8):
            pexp(r)

        # ---- 6. Z[i] = sum_m c_m E'[m, i] on the PE, chasing the exps
        for (co, cn) in _chunks(NT):
            zc_ps = ps_z.tile([1, 512], F32, name="zc_ps", tag="z")
            for j, (po_, pn) in enumerate(PT):
                nc.tensor.matmul(zc_ps[0:1, 0:cn],
                                 lhsT=cpartb[0:pn, j:j + 1],
                                 rhs=E_all[0:pn, j * NT + co:j * NT + co + cn],
                                 start=(j == 0), stop=(j == 6))
            zb = corS.tile([1, 512], F32, name="zb")
            nc.scalar.activation(zb[0:1, 0:cn], zc_ps[0:1, 0:cn], AF.Copy)
            nc.sync.dma_start(out_d[0:1, co:co + cn], zb[0:1, 0:cn])

        # ---- 7. qg/kg bias + l2 norm + lpos + queue negatives
        for br2, (b2, dstg) in enumerate([(c("bg2_sb"), qgT_bf),
                                          (c("bg2m_sb"), kgT_bf)]):
            qgT_f = gh.tile([128, 8], F32, name=f"qgT_f{br2}")
            nc.scalar.activation(qgT_f[:], garT[:, br2 * 8:br2 * 8 + 8],
                                 AF.Identity, bias=b2[:])
            sqg = gh.tile([128, 8], BF16, name=f"sqg{br2}")
            nc.scalar.activation(sqg[:], qgT_f[:], AF.Square)
            ssg_ps = ps_sim.tile([1, 8], F32, name="ssg_ps", tag="sim")
            nc.tensor.matmul(ssg_ps[:], lhsT=c("onescb_sb")[:],
                             rhs=sqg[:], start=True, stop=True)
            nrg = gh.tile([1, 8], F32, name=f"nrg{br2}")
            nc.vector.tensor_scalar_max(nrg[:], ssg_ps[:], 1e-12)
            nrg2 = gh.tile([1, 8], F32, name=f"nrg2{br2}")
            nc.scalar.activation(nrg2[:], nrg[:], AF.Ln)
            rng = gh.tile([1, 8], F32, name=f"rng{br2}")
            nc.scalar.activation(rng[:], nrg2[:], AF.Exp, scale=-0.5)
            rngb_ps = ps_sim.tile([128, 8], F32, name="rngb_ps", tag="sim")
            nc.tensor.matmul(rngb_ps[:], lhsT=c("onesr_sb")[:],
                             rhs=rng[:], start=True, stop=True)
            nc.vector.tensor_mul(dstg[:], qgT_f[:], rngb_ps[:])
        lpm = gh.tile([128, 8], F32, name="lpm")
        nc.vector.tensor_mul(lpm[:], qgT_bf[:], kgT_bf[:])
        lp_ps = ps_sim.tile([1, 8], F32, name="lp_ps", tag="sim")
        nc.tensor.matmul(lp_ps[:], lhsT=c("onesc_sb")[:], rhs=lpm[:],
                         start=True, stop=True)
        nc.scalar.activation(fin_sb[0:1, 1:9], lp_ps[:], AF.Copy)
        nc.sync.dma_start(out_d[0:1, NT + 9:NT + 17], fin_sb[0:1, 1:9])
        nc.vector.tensor_copy(qgT_f8[:], qgT_bf[:])
        # queue negatives: transposed orientation, single exp
        qe_ps = ps_z.tile([128, 512], F32, name="qe_ps", tag="z")
        for qt in range(64):
            nc.tensor.matmul(
                qe_ps[:, qt * 8:(qt + 1) * 8],
                lhsT=c("queueT_sb")[:, qt * 128:(qt + 1) * 128],
                rhs=qgT_f8[:], start=True, stop=True)
        qe_sb = gh.tile([128, 512], BF16, name="qe_sb")
        nc.scalar.activation(qe_sb[:], qe_ps[:], AF.Exp, scale=ISC)
        qs_ps = ps_sim.tile([1, 512], F32, name="qs_ps", tag="sim")
        for (o, n) in _chunks(512):
            nc.tensor.matmul(qs_ps[:, o:o + n], lhsT=c("onescb_sb")[:],
                             rhs=qe_sb[:, o:o + n], start=True, stop=True)
        qsum_r = gh.tile([1, 8], F32, name="qsum_r")
        nc.vector.reduce_sum(qsum_r[:],
                             qs_ps[:].rearrange("p (t i) -> p i t", i=8),
                             axis=mybir.AxisListType.X)
        nc.sync.dma_start(out_d[0:1, NT + 1:NT + 9], qsum_r[:])


def _prep_inputs(inputs):
    fq = np.asarray(inputs["feat_q"], np.float32).reshape(B, HW, C)
    fk = np.asarray(inputs["feat_k"], np.float32).reshape(B, HW, C)

    def xT(x):  # (784, 1024) -> (128, 8*784) fp8 with [c, ct*784+p]
        return np.ascontiguousarray(
            x.reshape(HW, CT, 128).transpose(2, 1, 0).reshape(128, CT * HW)
        ).astype(F8NP)

    def w1tile(w):  # (1024, 2048) -> (16, 128, 1024) fp8, scaled
        return np.ascontiguousarray(
            (w * WSCALE).reshape(CT, 128, DT, 128).transpose(2, 1, 0, 3)
            .reshape(DT, 128, C)).astype(F8NP)

    def w2tile(w):  # (2048, 128) -> (128, 2048) with [c, dt*128+d]
        return np.ascontiguousarray(
            w.reshape(DT, 128, 128).transpose(1, 0, 2).reshape(128, D)
        ).astype(BF)

    queue = np.asarray(inputs["queue"], np.float32)
    wg1 = np.asarray(inputs["Wg1"], np.float32)   # (1024, 2048)
    wg1m = np.asarray(inputs["mWg1"], np.float32)
    wg2 = np.asarray(inputs["Wg2"], np.float32)   # (2048, 128)
    wg2m = np.asarray(inputs["mWg2"], np.float32)
    bg1 = np.asarray(inputs["bg1"], np.float32)
    bg1m = np.asarray(inputs["mbg1"], np.float32)

    iotap = (np.arange(128, dtype=np.float32)[:, None]
             + 128.0 * np.arange(8, dtype=np.float32)[None, :])

    shared = {
        "wd1": w1tile(np.asarray(inputs["Wd1"], np.float32)),
        "wd1m": w1tile(np.asarray(inputs["mWd1"], np.float32)),
        "wd2": w2tile(np.asarray(inputs["Wd2"], np.float32)),
        "wd2m": w2tile(np.asarray(inputs["mWd2"], np.float32)),
        "bd1": np.ascontiguousarray(
            (np.asarray(inputs["bd1"], np.float32) * WSCALE)
            .reshape(DT, 128).T).astype(np.float32),
        "bd1m": np.ascontiguousarray(
            (np.asarray(inputs["mbd1"], np.float32) * WSCALE)
            .reshape(DT, 128).T).astype(np.float32),
        "bd2": (np.asarray(inputs["bd2"], np.float32) * WSCALE
                ).reshape(128, 1),
        "bd2m": (np.asarray(inputs["mbd2"], np.float32) * WSCALE
                 ).reshape(128, 1),
        "bg2": np.asarray(inputs["bg2"], np.float32).reshape(128, 1),
        "bg2m": np.asarray(inputs["mbg2"], np.float32).reshape(128, 1),
        "iotap": np.ascontiguousarray(iotap),
        "onesc": np.ones((128, 1), np.float32),
        "onesr": np.ones((1, 128), np.float32),
    }
    in_maps = []
    for cc in range(N_CORES):
        m = dict(shared)
        m["xq"] = xT(fq[cc])
        m["xk"] = xT(fk[cc])
        m["queueT"] = np.ascontiguousarray(
            queue[cc * QSH:(cc + 1) * QSH].T).astype(F8NP)
        # per-core D-slice of the global head: dts {2c, 2c+1}
        dsl = slice(cc * GDT * 128, (cc + 1) * GDT * 128)
        # wg1 slice layout [c, (ct*GDT+dl)*128 + d]
        m["wg1"] = np.ascontiguousarray(
            wg1[:, dsl].reshape(CT, 128, GDT * 128).transpose(1, 0, 2)
            .reshape(128, CT * GDT * 128)).astype(BF)
        m["wg1m"] = np.ascontiguousarray(
            wg1m[:, dsl].reshape(CT, 128, GDT * 128).transpose(1, 0, 2)
            .reshape(128, CT * GDT * 128)).astype(BF)
        # wg2 slice [dl*128+r, P] -> lhsT layout [r, dl*128+p]
        m["wg2"] = np.ascontiguousarray(
            wg2[dsl].reshape(GDT, 128, 128).transpose(1, 0, 2)
            .reshape(128, GDT * 128)).astype(BF)
        m["wg2m"] = np.ascontiguousarray(
            wg2m[dsl].reshape(GDT, 128, 128).transpose(1, 0, 2)
            .reshape(128, GDT * 128)).astype(BF)
        m["bg1"] = np.ascontiguousarray(
            bg1[dsl].reshape(GDT, 128).T).astype(np.float32)
        m["bg1m"] = np.ascontiguousarray(
            bg1m[dsl].reshape(GDT, 128).T).astype(np.float32)
        in_maps.append(m)
    return in_maps


_NC = None


def _get_nc():
    global _NC
    if _NC is None:
        _NC = _build()
    return _NC


def _host_combine(outs):
    """outs: [8, 1, OUTW] per-core partial rows -> final scalar loss.

    Per core: [0:6272] Z row-sum partials over its 784 logit columns,
    [6272] partial sum(max sim) over its rows, [6273:6281] partial
    sum(exp(l_neg/tau)) per image over its queue shard, [6281:6289]
    l_pos per image (replicated).
    """
    outs = np.asarray(outs, np.float64).reshape(len(outs), -1)
    Zf = outs[:, 0:NT].sum(axis=0)
    possum = outs[:, NT].sum()
    l_d = np.mean(np.log(Zf)) - ISC * possum / NT
    qsums = outs[:, NT + 1:NT + 9].sum(axis=0)
    lpos = outs[0, NT + 9:NT + 17]
    lse = np.log(np.exp(ISC * lpos) + qsums)
    l_g = np.mean(lse - ISC * lpos)
    return np.float32((1.0 - LAM) * l_g + LAM * l_d).reshape(())


def kernel(**inputs) -> np.ndarray:
    nc = _get_nc()
    in_maps = _prep_inputs(inputs)
    res = bass_utils.run_bass_kernel_spmd(nc, in_maps,
                                          core_ids=list(range(N_CORES)))
    outs = np.stack([res.results[c]["out"].reshape(1, OUTW)
                     for c in range(N_CORES)])
    return _host_combine(outs)


# revision 30
# speedup vs baseline: 1.1224x; 1.0383x over previous
"""DenseCL loss kernel for 8 TRN2 NeuronCores.

Sharding: core c owns batch image c for the dense branch, queue rows
[c*8192, (c+1)*8192) for the queue-InfoNCE negatives, and the COLUMN block
[c*784, (c+1)*784) of the flat dense-InfoNCE logits.

Key identity: matched_k[j] = k_d[:, idx_j], so the dense logits matrix is a
column gather of P = k_d_local^T @ q_all.  Each core computes partial row
sums Z_i = sum_m c_m * exp(P[m, i] / tau) where c is the histogram of its
own argmax indices (the weighted partition sum runs on the PE with the
counts as a stationary column), and the positives are the sim row maxima.
No matched-key gather and no matched-key AllGather is needed.

Collectives (gpsimd stream): a dummy 32-byte AllGather issued first thing
absorbs the cross-core start-skew barrier; then AllGather of pooled
features, AllGather of normalized q_d (fp8 bytes moved as f32 elements,
hidden under the k branch), AllReduce of the D-sharded global-head
partials.  Final ~10K-flop unshard happens on the host.
"""
import os
import sys

if "/opt/trn_rl_repo" not in sys.path:
    sys.path.insert(0, "/opt/trn_rl_repo")

USE_DR = os.environ.get("KDR", "1") == "1"      # fp8 DoubleRow for dense L1

import numpy as np
import ml_dtypes

import concourse.bass as bass
import concourse.bacc as bacc
import concourse.mybir as mybir
import concourse.tile as tile
from concourse import bass_utils, masks

BF = ml_dtypes.bfloat16
F8NP = ml_dtypes.float8_e4m3
F32 = mybir.dt.float32
BF16 = mybir.dt.bfloat16
F8 = mybir.dt.float8e4
DR = mybir.MatmulPerfMode.DoubleRow

N_CORES = 8
B, HW, C, D, P, Q = 8, 784, 1024, 2048, 128, 65536
QSH = Q // N_CORES          # 8192 queue rows per core
CT, DT = C // 128, D // 128  # 8, 16
GDT = DT // N_CORES         # 2 ghead D-tiles per core
NT = B * HW                 # 6272 total dense rows
TAU = 0.2
LAM = 0.5
ISC = 1.0 / TAU             # 5.0
WSCALE = 32.0               # fp8 range scale for W1/b1 (cancelled by l2 norm)
AF = mybir.ActivationFunctionType
ALU = mybir.AluOpType

# 784 = 6*128 + 16 partition tiles
PT = [(i * 128, min(128, HW - i * 128)) for i in range(7)]
OUTW = 8192                 # out row: [0:6272] Z, 6272 possum,
                            # [6273:6281] qsums, [6281:6289] lpos


def _chunks(n, step=512):
    return [(o, min(step, n - o)) for o in range(0, n, step)]


def _patch_act_tables():
    """Force every activation we use onto the natural_log_exp_and_others
    table set so the kernel needs exactly one ACT_TABLE_LOAD."""
    from concourse import hw_specs
    import concourse.bacc as bacc_mod
    if getattr(bacc_mod, "_act_tables_patched", False):
        return
    orig = hw_specs.get_activation_tables
    ours = {AF.Exp, AF.Ln, AF.Relu, AF.Identity, AF.Copy, AF.Square}
    keep = "natural_log_exp_and_others"

    def patched(arch):
        tabs = orig(arch)
        assert keep in tabs and ours <= tabs[keep]
        return {name: (fns if name == keep else fns - ours)
                for name, fns in tabs.items()}

    bacc_mod.get_activation_tables = patched
    bacc_mod._act_tables_patched = True


def _build(do_compile=True):
    _patch_act_tables()
    nc = bacc.Bacc("TRN2", target_bir_lowering=False, debug=False,
                   num_devices=N_CORES)

    def inp(name, shape, dt):
        return nc.dram_tensor(name, list(shape), dt, kind="ExternalInput")

    xq_d = inp("xq", (128, CT * HW), F8)          # [c, ct*784+p] = feat_q[b, p, ct*128+c]
    xk_d = inp("xk", (128, CT * HW), F8)
    wd1_d = inp("wd1", (DT, 128, C), F8)          # [dt, c, ct*128+d] = 32*Wd1[ct*128+c, dt*128+d]
    wd1m_d = inp("wd1m", (DT, 128, C), F8)
    wd2_d = inp("wd2", (128, D), BF16)            # [c, dt*128+d] = Wd2[dt*128+c, d]
    wd2m_d = inp("wd2m", (128, D), BF16)
    wg1_d = inp("wg1", (128, CT * GDT * 128), BF16)  # per-core D-slice of Wg1
    wg1m_d = inp("wg1m", (128, CT * GDT * 128), BF16)
    wg2_d = inp("wg2", (128, GDT * 128), BF16)    # per-core D-slice of Wg2 (lhsT)
    wg2m_d = inp("wg2m", (128, GDT * 128), BF16)
    bd1_d = inp("bd1", (128, DT), F32)            # [r, dt] = 32*bd1[dt*128+r]
    bd1m_d = inp("bd1m", (128, DT), F32)
    bd2_d = inp("bd2", (128, 1), F32)             # 32*bd2
    bd2m_d = inp("bd2m", (128, 1), F32)
    bg1_d = inp("bg1", (128, GDT), F32)           # per-core D-slice of bg1
    bg1m_d = inp("bg1m", (128, GDT), F32)
    bg2_d = inp("bg2", (128, 1), F32)
    bg2m_d = inp("bg2m", (128, 1), F32)
    queueT_d = inp("queueT", (128, QSH), F8)      # [ch, j] = queue[c0+j, ch]
    iotap_d = inp("iotap", (128, 8), F32)         # col i = p + 128*i
    onesc_d = inp("onesc", (128, 1), F32)         # ones column (lhsT partition sums)
    onesr_d = inp("onesr", (1, 128), F32)         # ones row (lhsT for K=1 broadcast)

    out_d = nc.dram_tensor("out", [1, OUTW], F32, kind="ExternalOutput")

    with tile.TileContext(nc) as tc:
        rg = [list(range(N_CORES))]
        with tc.tile_pool(name="dramp", bufs=1, space="DRAM") as dpool:
            pool_in = dpool.tile([2 * C], F32, name="pool_in")
            pool_out = dpool.tile([N_CORES * 2 * C], F32, name="pool_out",
                                  addr_space="Shared")
            # q_d fp8 bytes shipped as f32 elements (4x fewer CCE elements)
            qd_in = dpool.tile([128 * HW // 4], F32, name="qd_in")
            qd_out = dpool.tile([N_CORES * 128 * HW // 4], F32, name="qd_out",
                                addr_space="Shared")
            gar_in = dpool.tile([128 * 16], F32, name="gar_in")
            gar_out = dpool.tile([128 * 16], F32, name="gar_out",
                                 addr_space="Shared")
            _body(nc, tc, rg, locals())
    if do_compile:
        nc.compile()
    return nc


def _body(nc, tc, rg, env):
    g = lambda k: env[k]

    with tc.tile_pool(name="cst", bufs=1) as cst:

        def load(name, shape, dt, eng=None):
            t = cst.tile(list(shape), dt, name=name + "_sb")
            (eng or nc.sync).dma_start(t[:], g(name + "_d")[:])
            return t

        iotap_sb = load("iotap", (128, 8), F32, eng=nc.gpsimd)
        onesc_sb = load("onesc", (128, 1), F32, eng=nc.gpsimd)
        onesr_sb = load("onesr", (1, 128), F32, eng=nc.gpsimd)

        # ---- sync ring: q-branch critical inputs (per-dt weight slices)
        bd1_sb = load("bd1", (128, DT), F32)
        bd2_sb = load("bd2", (128, 1), F32)
        xq_sb = cst.tile([128, CT * HW], F8, name="xq_sb")
        nc.sync.dma_start(xq_sb[:, 0:4 * HW], g("xq_d")[:, 0:4 * HW])
        wq1_sb = cst.tile([128, DT * C], F8, name="wq1_sb")
        wd2_sb = cst.tile([128, D], BF16, name="wd2_sb")
        for dt in range(DT):
            nc.sync.dma_start(wq1_sb[:, dt * C:(dt + 1) * C],
                              g("wd1_d")[dt, :, :])
            if dt == 0:
                nc.sync.dma_start(xq_sb[:, 4 * HW:CT * HW],
                                  g("xq_d")[:, 4 * HW:CT * HW])
            if dt == 2:
                nc.sync.dma_start(wd2_sb[:], g("wd2_d")[:])
        bd1m_sb = load("bd1m", (128, DT), F32)
        bd2m_sb = load("bd2m", (128, 1), F32)

        # ---- gpsimd (SWDGE) ring: k-branch + tail inputs, so the scalar
        # queue carries only ACT work (DMA triggers head-of-line-block an
        # engine queue once the ring fills)
        xk_sb = cst.tile([128, CT * HW], F8, name="xk_sb")
        nc.gpsimd.dma_start(xk_sb[:], g("xk_d")[:])
        wk1_sb = cst.tile([128, DT * C], F8, name="wk1_sb")
        for dt in range(DT):
            nc.gpsimd.dma_start(wk1_sb[:, dt * C:(dt + 1) * C],
                                g("wd1m_d")[dt, :, :])
        wd2m_sb = load("wd2m", (128, D), BF16, eng=nc.gpsimd)

        onescb_sb = cst.tile([128, 1], BF16, name="onescb_sb")
        nc.vector.tensor_copy(onescb_sb[:], onesc_sb[:])
        id_f = cst.tile([128, 128], F32, name="id_f")
        masks.make_identity(nc, id_f[:])

        # long-lived results
        qdT_bf = cst.tile([128, HW], BF16, name="qdT_bf")
        kdT_bf = cst.tile([128, HW], BF16, name="kdT_bf")
        kdT_f8 = cst.tile([128, HW], F8, name="kdT_f8")
        qdT_f8 = cst.tile([128, HW], F8, name="qdT_f8")
        qall_sb = cst.tile([128, NT], F8, name="qall_sb")
        qgT_bf = cst.tile([128, 8], BF16, name="qgT_bf")
        kgT_bf = cst.tile([128, 8], BF16, name="kgT_bf")
        qgT_f8 = cst.tile([128, 8], F8, name="qgT_f8")
        pool_sb = cst.tile([128, 16], F32, name="pool_sb")
        gqall = cst.tile([128, 64], F32, name="gqall")  # pooled q [c, (r t)]
        gkall = cst.tile([128, 64], F32, name="gkall")
        fin_sb = cst.tile([1, 16], F32, name="fin_sb")

        ctx = dict(locals())
        _dense(nc, tc, rg, env, cst, ctx)
        _tail(nc, tc, rg, env, cst, ctx)


def _dense(nc, tc, rg, env, cst, ctx):
    g = lambda k: env[k]
    c = lambda k: ctx[k]
    pool_in, pool_out = g("pool_in"), g("pool_out")
    qd_in, qd_out = g("qd_in"), g("qd_out")
    xq_sb, xk_sb = c("xq_sb"), c("xk_sb")
    pool_sb = c("pool_sb")

    with tc.tile_pool(name="hp", bufs=3) as hp, \
         tc.tile_pool(name="l2s", bufs=2) as l2s, \
         tc.tile_pool(name="plp", bufs=2) as plp, \
         tc.tile_pool(name="ps_big", bufs=2, space="PSUM") as ps_big, \
         tc.tile_pool(name="ps_qd", bufs=2, space="PSUM") as ps_qd:

        def dense_branch(br, xs, w1sb, w2sb, b1, b2, dst, dst8):
            qd_ps = ps_qd.tile([128, HW], F32, name="qd_ps", tag="qd")
            for dt in range(DT):
                w1t = w1sb[:, dt * C:(dt + 1) * C]
                h_ps = ps_big.tile([128, HW], F32, name="h_ps", tag="big")
                if USE_DR:
                    for cp in range(CT // 2):
                        wp = w1t[:, cp * 256:(cp + 1) * 256].rearrange(
                            "p (two m) -> p two m", two=2)
                        xp = xs[:, cp * 2 * HW:(cp + 1) * 2 * HW].rearrange(
                            "p (two n) -> p two n", two=2)
                        for (o, n) in _chunks(HW):
                            nc.tensor.matmul(
                                h_ps[:, o:o + n],
                                lhsT=wp,
                                rhs=xp[:, :, o:o + n],
                                start=(cp == 0), stop=(cp == CT // 2 - 1),
                                perf_mode=DR)
                else:
                    for ct in range(CT):
                        for (o, n) in _chunks(HW):
                            nc.tensor.matmul(
                                h_ps[:, o:o + n],
                                lhsT=w1t[:, ct * 128:(ct + 1) * 128],
                                rhs=xs[:, ct * HW + o:ct * HW + o + n],
                                start=(ct == 0), stop=(ct == CT - 1))
                h_sb = hp.tile([128, HW], BF16, name="h_sb")
                nc.scalar.activation(h_sb[:], h_ps[:], AF.Relu,
                                     bias=b1[:, dt:dt + 1])
                if br == 0 and dt in (2, 4, 6, 8):
                    # pooling of xq/xk on DVE while PE grinds L1
                    base = 0 if dt in (2, 4) else 8
                    src = xq_sb if dt in (2, 4) else xk_sb
                    c0 = 0 if dt in (2, 6) else 4
                    for ct2 in range(c0, c0 + 4):
                        scr = plp.tile([128, HW], BF16, name="pool_scr")
                        nc.vector.tensor_scalar(
                            scr[:], src[:, ct2 * HW:(ct2 + 1) * HW], 1.0,
                            None, op0=ALU.mult, op1=ALU.add,
                            accum_out=pool_sb[:, base + ct2:base + ct2 + 1])
                # L2 for the PREVIOUS dt: its relu finished during this
                # dt's L1 matmuls, so the PE never waits on the ACT chain
                if dt > 0:
                    for (o, n) in _chunks(HW):
                        nc.tensor.matmul(
                            qd_ps[:, o:o + n],
                            lhsT=w2sb[:, (dt - 1) * 128:dt * 128],
                            rhs=h_prev[:, o:o + n],
                            start=(dt == 1), stop=False)
                h_prev = h_sb
            for (o, n) in _chunks(HW):
                nc.tensor.matmul(
                    qd_ps[:, o:o + n],
                    lhsT=w2sb[:, (DT - 1) * 128:DT * 128],
                    rhs=h_prev[:, o:o + n],
                    start=False, stop=True)
            # bias + l2-normalize along channels (partition dim)
            qdT_f = l2s.tile([128, HW], F32, name="qdT_f")
            nc.scalar.activation(qdT_f[:], qd_ps[:], AF.Identity, bias=b2[:])
            sq = l2s.tile([128, HW], BF16, name="sq")
            nc.scalar.activation(sq[:], qdT_f[:], AF.Square)
            ssq_ps = ps_qd.tile([1, HW], F32, name="ssq_ps", tag="qd")
            for (o, n) in _chunks(HW):
                nc.tensor.matmul(ssq_ps[:, o:o + n], lhsT=c("onescb_sb")[:],
                                 rhs=sq[:, o:o + n], start=True, stop=True)
            nrm = l2s.tile([1, HW], F32, name="nrm")
            nc.vector.tensor_scalar_max(nrm[:], ssq_ps[:], 1e-12)
            # rsqrt(s) = exp(-0.5*ln(s)) keeps ACT on one table set
            nrm2 = l2s.tile([1, HW], F32, name="nrm2")
            nc.scalar.activation(nrm2[:], nrm[:], AF.Ln)
            rn = l2s.tile([1, HW], F32, name="rn")
            nc.scalar.activation(rn[:], nrm2[:], AF.Exp, scale=-0.5)
            rnb_ps = ps_qd.tile([128, HW], F32, name="rnb_ps", tag="qd")
            for (o, n) in _chunks(HW):
                nc.tensor.matmul(rnb_ps[:, o:o + n], lhsT=c("onesr_sb")[:],
                                 rhs=rn[:, o:o + n], start=True, stop=True)
            nc.vector.tensor_mul(dst[:], qdT_f[:], rnb_ps[:])
            nc.vector.tensor_copy(dst8[:], dst[:])

        dense_branch(0, xq_sb, c("wq1_sb"), c("wd2_sb"), c("bd1_sb"),
                     c("bd2_sb"), c("qdT_bf"), c("qdT_f8"))
        # ship q_d: this collective gates the whole logits tail, so it
        # goes first on the collective stream
        nc.sync.dma_start(
            qd_in[:].rearrange("(c p) -> c p", c=128),
            c("qdT_f8")[:].bitcast(F32))
        nc.gpsimd.collective_compute(
            "AllGather", ALU.bypass, replica_groups=rg,
            ins=[qd_in.opt()], outs=[qd_out.opt()])
        # tail inputs ride the gpsimd ring while it is blocked on the AG
        queueT_sb = cst.tile([128, QSH], F8, name="queueT_sb")
        nc.gpsimd.dma_start(queueT_sb[:], g("queueT_d")[:])
        ctx["queueT_sb"] = queueT_sb
        for nm in ("bg1", "bg1m", "bg2", "bg2m"):
            t = cst.tile([128, GDT] if nm in ("bg1", "bg1m") else [128, 1],
                         F32, name=nm + "_sb")
            nc.gpsimd.dma_start(t[:], g(nm + "_d")[:])
            ctx[nm + "_sb"] = t
        for nm, w in (("wg1", CT * GDT * 128), ("wg1m", CT * GDT * 128),
                      ("wg2", GDT * 128), ("wg2m", GDT * 128)):
            t = cst.tile([128, w], BF16, name=nm + "_sb")
            nc.gpsimd.dma_start(t[:], g(nm + "_d")[:])
            ctx[nm + "_sb"] = t
        # pooled features AllGather (second on the stream)
        pin = pool_in[:].rearrange("(g t c) -> c (g t)", g=2, t=8, c=128)
        nc.gpsimd.dma_start(pin, pool_sb[:])
        nc.gpsimd.collective_compute(
            "AllGather", ALU.bypass, replica_groups=rg,
            ins=[pool_in.opt()], outs=[pool_out.opt()])
        # qall load split across the sync and tensor rings
        for r in range(8):
            nc.sync.dma_start(
                c("qall_sb")[:, r * HW:(r + 1) * HW].bitcast(F32),
                qd_out[r * 128 * HW // 4:(r + 1) * 128 * HW // 4]
                .rearrange("(c p) -> c p", c=128))
        # pooled features for every image: [c, (r t)] layout
        pg = pool_out[:].rearrange("(r g x) -> r g x", r=8, g=2)
        for gi, dstp in ((0, c("gqall")), (1, c("gkall"))):
            for r in range(8):
                nc.sync.dma_start(
                    dstp[:, r * 8:(r + 1) * 8],
                    pg[r, gi, :].rearrange("(t c) -> c t", c=128))

        dense_branch(1, xk_sb, c("wk1_sb"), c("wd2m_sb"), c("bd1m_sb"),
                     c("bd2m_sb"), c("kdT_bf"), c("kdT_f8"))


def _tail(nc, tc, rg, env, cst, ctx):
    """Everything after the dense branches, emission-ordered so no engine
    queue ever head-of-line-blocks on late data:

      sim -> argmax(DVE, concurrent) -> P/exp chunks 0-3 -> histogram ->
      ghead partials + AllReduce -> P/exp chunks 4-7 -> Z (PE, chasing) ->
      qg norm + lpos + queue negatives.
    """
    g = lambda k: env[k]
    c = lambda k: ctx[k]
    out_d = g("out_d")
    gar_in, gar_out = g("gar_in"), g("gar_out")
    qdT_bf, kdT_bf = c("qdT_bf"), c("kdT_bf")
    qall_sb, kdT_f8 = c("qall_sb"), c("kdT_f8")
    fin_sb = c("fin_sb")
    qgT_bf, kgT_bf, qgT_f8 = c("qgT_bf"), c("kgT_bf"), c("qgT_f8")

    with tc.tile_pool(name="cor", bufs=1) as cor, \
         tc.tile_pool(name="corS", bufs=2) as corS, \
         tc.tile_pool(name="lg", bufs=1) as lgp, \
         tc.tile_pool(name="gh", bufs=1) as gh, \
         tc.tile_pool(name="ps_lg", bufs=2, space="PSUM") as ps_lg, \
         tc.tile_pool(name="ps_sim", bufs=1, space="PSUM") as ps_sim, \
         tc.tile_pool(name="ps_ir", bufs=1, space="PSUM") as ps_ir, \
         tc.tile_pool(name="ps_z", bufs=1, space="PSUM") as ps_z:

        # ---- 1. sim + argmax (argmax chain runs on DVE concurrent with
        # the P/exp chunks below)
        sim_sb = cor.tile([128, 7 * HW], BF16, name="sim_sb")
        mx8 = cor.tile([128, 8], BF16, name="mx8")
        ix8 = cor.tile([128, 8], mybir.dt.uint32, name="ix8")
        ixf = cor.tile([128, 7], F32, name="ixf")
        posv = cor.tile([128, 7], F32, name="posv")
        nc.vector.memset(posv[:], 0.0)
        for i, (po_, pn) in enumerate(PT):
            s_ps = ps_sim.tile([128, HW], F32, name="s_ps", tag="sim")
            for (o, n) in _chunks(HW):
                nc.tensor.matmul(s_ps[0:pn, o:o + n],
                                 lhsT=qdT_bf[:, po_:po_ + pn],
                                 rhs=kdT_bf[:, o:o + n],
                                 start=True, stop=True)
            nc.scalar.activation(sim_sb[0:pn, i * HW:i * HW + HW],
                                 s_ps[0:pn, :], AF.Copy)
            nc.vector.max(mx8[0:pn, :], sim_sb[0:pn, i * HW:i * HW + HW])
            nc.vector.max_index(ix8[0:pn, :], mx8[0:pn, :],
                                sim_sb[0:pn, i * HW:i * HW + HW])
            nc.vector.tensor_copy(ixf[0:pn, i:i + 1], ix8[0:pn, 0:1])
            nc.vector.tensor_copy(posv[0:pn, i:i + 1], mx8[0:pn, 0:1])

        # ---- 2. P/exp, per-rank chunks (column-sharded logits)
        E_all = lgp.tile([128, 7 * NT], BF16, name="E_all")
        cpartb = cor.tile([128, 7], BF16, name="cpartb")

        def pexp(r):
            co = r * HW
            for j, (po_, pn) in enumerate(PT):
                lg_ps = ps_lg.tile([128, HW], F32, name="lg_ps")
                for (o, n) in _chunks(HW):
                    nc.tensor.matmul(
                        lg_ps[0:pn, o:o + n],
                        lhsT=kdT_f8[:, po_:po_ + pn],
                        rhs=qall_sb[:, co + o:co + o + n],
                        start=True, stop=True)
                nc.scalar.activation(E_all[0:pn, j * NT + co:j * NT + co + HW],
                                     lg_ps[0:pn, :], AF.Exp, scale=ISC)

        for r in range(4):
            pexp(r)

        # ---- 3. histogram of argmax indices + positives partial
        ir_sb = cor.tile([1, HW], F32, name="ir_sb")
        for i, (po_, pn) in enumerate(PT):
            ir_ps = ps_ir.tile([1, 128], F32, name="ir_ps", tag="ir")
            nc.tensor.transpose(ir_ps[0:1, 0:pn], ixf[0:pn, i:i + 1],
                                c("id_f")[0:pn, 0:pn])
            nc.scalar.activation(ir_sb[0:1, po_:po_ + pn],
                                 ir_ps[0:1, 0:pn], AF.Copy)
        ib_ps = ps_sim.tile([128, HW], F32, name="ib_ps", tag="sim")
        for (o, n) in _chunks(HW):
            nc.tensor.matmul(ib_ps[:, o:o + n], lhsT=c("onesr_sb")[:],
                             rhs=ir_sb[:, o:o + n], start=True, stop=True)
        ib_sb = cor.tile([128, HW], F32, name="ib_sb")
        nc.scalar.activation(ib_sb[:], ib_ps[:], AF.Copy)
        cpart = cor.tile([128, 7], F32, name="cpart")
        nc.vector.memset(cpart[:], 0.0)
        for i, (po_, pn) in enumerate(PT):
            S = corS.tile([128, HW], BF16, name="S")
            nc.vector.tensor_scalar(
                S[0:pn, :], ib_sb[0:pn, :], c("iotap_sb")[0:pn, i:i + 1],
                None, op0=ALU.is_equal, op1=ALU.add,
                accum_out=cpart[0:pn, i:i + 1])
        nc.vector.tensor_copy(cpartb[:], cpart[:])
        pos_ps = ps_ir.tile([1, 128], F32, name="pos_ps", tag="ir")
        nc.tensor.matmul(pos_ps[0:1, 0:7], lhsT=c("onesc_sb")[:],
                         rhs=posv[:], start=True, stop=True)
        nc.vector.reduce_sum(fin_sb[0:1, 0:1], pos_ps[0:1, 0:7],
                             axis=mybir.AxisListType.X)
        nc.sync.dma_start(out_d[0:1, NT:NT + 1], fin_sb[0:1, 0:1])

        # ---- 4. global-head partials (D-sharded) + AllReduce
        gq_bf = gh.tile([128, 64], BF16, name="gq_bf")
        gk_bf = gh.tile([128, 64], BF16, name="gk_bf")
        nc.vector.tensor_scalar_mul(gq_bf[:], c("gqall")[:], 1.0 / HW)
        nc.vector.tensor_scalar_mul(gk_bf[:], c("gkall")[:], 1.0 / HW)
        gprt = gh.tile([128, 16], F32, name="gprt")
        for br2, (gsb, w1sb, w2sb, b1c) in enumerate([
                (gq_bf, c("wg1_sb"), c("wg2_sb"), c("bg1_sb")),
                (gk_bf, c("wg1m_sb"), c("wg2m_sb"), c("bg1m_sb"))]):
            gv = gsb[:].rearrange("c (r t) -> c t r", t=8)
            qg_ps = ps_z.tile([128, 8], F32, name="qg_ps", tag="z")
            for dl in range(GDT):
                hgt_ps = ps_ir.tile([128, 8], F32, name="hgt_ps", tag="ir")
                for ct in range(CT):
                    nc.tensor.matmul(
                        hgt_ps[:],
                        lhsT=w1sb[:, (ct * GDT + dl) * 128:
                                  (ct * GDT + dl + 1) * 128],
                        rhs=gv[:, ct, :],
                        start=(ct == 0), stop=(ct == CT - 1))
                hgt_sb = gh.tile([128, 8], BF16, name=f"hgt{br2}_{dl}")
                nc.vector.tensor_scalar(hgt_sb[:], hgt_ps[:],
                                        b1c[:, dl:dl + 1], 0.0,
                                        op0=ALU.add, op1=ALU.max)
                nc.tensor.matmul(qg_ps[:],
                                 lhsT=w2sb[:, dl * 128:(dl + 1) * 128],
                                 rhs=hgt_sb[:], start=(dl == 0),
                                 stop=(dl == GDT - 1))
            nc.vector.tensor_copy(gprt[:, br2 * 8:br2 * 8 + 8], qg_ps[:])
        nc.gpsimd.dma_start(
            gar_in[:].rearrange("(c p) -> c p", c=128), gprt[:])
        nc.gpsimd.collective_compute(
            "AllReduce", ALU.add, replica_groups=rg,
            ins=[gar_in.opt()], outs=[gar_out.opt()])
        garT = gh.tile([128, 16], F32, name="garT")
        nc.gpsimd.dma_start(
            garT[:], gar_out[:].rearrange("(c p) -> c p", c=128))

        # ---- 5. remaining P/exp chunks
        for r in range(4, 8):
            pexp(r)

        # ---- 6. Z[i] = sum_m c_m E'[m, i] on the PE, chasing the exps
        for (co, cn) in _chunks(NT):
            zc_ps = ps_z.tile([1, 512], F32, name="zc_ps", tag="z")
            for j, (po_, pn) in enumerate(PT):
                nc.tensor.matmul(zc_ps[0:1, 0:cn],
                                 lhsT=cpartb[0:pn, j:j + 1],
                                 rhs=E_all[0:pn, j * NT + co:j * NT + co + cn],
                                 start=(j == 0), stop=(j == 6))
            zb = corS.tile([1, 512], F32, name="zb")
            nc.scalar.activation(zb[0:1, 0:cn], zc_ps[0:1, 0:cn], AF.Copy)
            nc.sync.dma_start(out_d[0:1, co:co + cn], zb[0:1, 0:cn])

        # ---- 7. qg/kg bias + l2 norm + lpos + queue negatives
        for br2, (b2, dstg) in enumerate([(c("bg2_sb"), qgT_bf),
                                          (c("bg2m_sb"), kgT_bf)]):
            qgT_f = gh.tile([128, 8], F32, name=f"qgT_f{br2}")
            nc.scalar.activation(qgT_f[:], garT[:, br2 * 8:br2 * 8 + 8],
                                 AF.Identity, bias=b2[:])
            sqg = gh.tile([128, 8], BF16, name=f"sqg{br2}")
            nc.scalar.activation(sqg[:], qgT_f[:], AF.Square)
            ssg_ps = ps_sim.tile([1, 8], F32, name="ssg_ps", tag="sim")
            nc.tensor.matmul(ssg_ps[:], lhsT=c("onescb_sb")[:],
                             rhs=sqg[:], start=True, stop=True)
            nrg = gh.tile([1, 8], F32, name=f"nrg{br2}")
            nc.vector.tensor_scalar_max(nrg[:], ssg_ps[:], 1e-12)
            nrg2 = gh.tile([1, 8], F32, name=f"nrg2{br2}")
            nc.scalar.activation(nrg2[:], nrg[:], AF.Ln)
            rng = gh.tile([1, 8], F32, name=f"rng{br2}")
            nc.scalar.activation(rng[:], nrg2[:], AF.Exp, scale=-0.5)
            rngb_ps = ps_sim.tile([128, 8], F32, name="rngb_ps", tag="sim")
            nc.tensor.matmul(rngb_ps[:], lhsT=c("onesr_sb")[:],
                             rhs=rng[:], start=True, stop=True)
            nc.vector.tensor_mul(dstg[:], qgT_f[:], rngb_ps[:])
        lpm = gh.tile([128, 8], F32, name="lpm")
        nc.vector.tensor_mul(lpm[:], qgT_bf[:], kgT_bf[:])
        lp_ps = ps_sim.tile([1, 8], F32, name="lp_ps", tag="sim")
        nc.tensor.matmul(lp_ps[:], lhsT=c("onesc_sb")[:], rhs=lpm[:],
                         start=True, stop=True)
        nc.scalar.activation(fin_sb[0:1, 1:9], lp_ps[:], AF.Copy)
        nc.sync.dma_start(out_d[0:1, NT + 9:NT + 17], fin_sb[0:1, 1:9])
        nc.vector.tensor_copy(qgT_f8[:], qgT_bf[:])
        # queue negatives: transposed orientation, single exp
        qe_ps = ps_z.tile([128, 512], F32, name="qe_ps", tag="z")
        for qt in range(64):
            nc.tensor.matmul(
                qe_ps[:, qt * 8:(qt + 1) * 8],
                lhsT=c("queueT_sb")[:, qt * 128:(qt + 1) * 128],
                rhs=qgT_f8[:], start=True, stop=True)
        qe_sb = gh.tile([128, 512], BF16, name="qe_sb")
        nc.scalar.activation(qe_sb[:], qe_ps[:], AF.Exp, scale=ISC)
        qs_ps = ps_sim.tile([1, 512], F32, name="qs_ps", tag="sim")
        for (o, n) in _chunks(512):
            nc.tensor.matmul(qs_ps[:, o:o + n], lhsT=c("onescb_sb")[:],
                             rhs=qe_sb[:, o:o + n], start=True, stop=True)
        qsum_r = gh.tile([1, 8], F32, name="qsum_r")
        nc.vector.reduce_sum(qsum_r[:],
                             qs_ps[:].rearrange("p (t i) -> p i t", i=8),
                             axis=mybir.AxisListType.X)
        nc.sync.dma_start(out_d[0:1, NT + 1:NT + 9], qsum_r[:])


def _prep_inputs(inputs):
    fq = np.asarray(inputs["feat_q"], np.float32).reshape(B, HW, C)
    fk = np.asarray(inputs["feat_k"], np.float32).reshape(B, HW, C)

    def xT(x):  # (784, 1024) -> (128, 8*784) fp8 with [c, ct*784+p]
        return np.ascontiguousarray(
            x.reshape(HW, CT, 128).transpose(2, 1, 0).reshape(128, CT * HW)
        ).astype(F8NP)

    def w1tile(w):  # (1024, 2048) -> (16, 128, 1024) fp8, scaled
        return np.ascontiguousarray(
            (w * WSCALE).reshape(CT, 128, DT, 128).transpose(2, 1, 0, 3)
            .reshape(DT, 128, C)).astype(F8NP)

    def w2tile(w):  # (2048, 128) -> (128, 2048) with [c, dt*128+d]
        return np.ascontiguousarray(
            w.reshape(DT, 128, 128).transpose(1, 0, 2).reshape(128, D)
        ).astype(BF)

    queue = np.asarray(inputs["queue"], np.float32)
    wg1 = np.asarray(inputs["Wg1"], np.float32)   # (1024, 2048)
    wg1m = np.asarray(inputs["mWg1"], np.float32)
    wg2 = np.asarray(inputs["Wg2"], np.float32)   # (2048, 128)
    wg2m = np.asarray(inputs["mWg2"], np.float32)
    bg1 = np.asarray(inputs["bg1"], np.float32)
    bg1m = np.asarray(inputs["mbg1"], np.float32)

    iotap = (np.arange(128, dtype=np.float32)[:, None]
             + 128.0 * np.arange(8, dtype=np.float32)[None, :])

    shared = {
        "wd1": w1tile(np.asarray(inputs["Wd1"], np.float32)),
        "wd1m": w1tile(np.asarray(inputs["mWd1"], np.float32)),
        "wd2": w2tile(np.asarray(inputs["Wd2"], np.float32)),
        "wd2m": w2tile(np.asarray(inputs["mWd2"], np.float32)),
        "bd1": np.ascontiguousarray(
            (np.asarray(inputs["bd1"], np.float32) * WSCALE)
            .reshape(DT, 128).T).astype(np.float32),
        "bd1m": np.ascontiguousarray(
            (np.asarray(inputs["mbd1"], np.float32) * WSCALE)
            .reshape(DT, 128).T).astype(np.float32),
        "bd2": (np.asarray(inputs["bd2"], np.float32) * WSCALE
                ).reshape(128, 1),
        "bd2m": (np.asarray(inputs["mbd2"], np.float32) * WSCALE
                 ).reshape(128, 1),
        "bg2": np.asarray(inputs["bg2"], np.float32).reshape(128, 1),
        "bg2m": np.asarray(inputs["mbg2"], np.float32).reshape(128, 1),
        "iotap": np.ascontiguousarray(iotap),
        "onesc": np.ones((128, 1), np.float32),
        "onesr": np.ones((1, 128), np.float32),
    }
    in_maps = []
    for cc in range(N_CORES):
        m = dict(shared)
        m["xq"] = xT(fq[cc])
        m["xk"] = xT(fk[cc])
        m["queueT"] = np.ascontiguousarray(
            queue[cc * QSH:(cc + 1) * QSH].T).astype(F8NP)
        # per-core D-slice of the global head: dts {2c, 2c+1}
        dsl = slice(cc * GDT * 128, (cc + 1) * GDT * 128)
        # wg1 slice layout [c, (ct*GDT+dl)*128 + d]
        m["wg1"] = np.ascontiguousarray(
            wg1[:, dsl].reshape(CT, 128, GDT * 128).transpose(1, 0, 2)
            .reshape(128, CT * GDT * 128)).astype(BF)
        m["wg1m"] = np.ascontiguousarray(
            wg1m[:, dsl].reshape(CT, 128, GDT * 128).transpose(1, 0, 2)
            .reshape(128, CT * GDT * 128)).astype(BF)
        # wg2 slice [dl*128+r, P] -> lhsT layout [r, dl*128+p]
        m["wg2"] = np.ascontiguousarray(
            wg2[dsl].reshape(GDT, 128, 128).transpose(1, 0, 2)
            .reshape(128, GDT * 128)).astype(BF)
        m["wg2m"] = np.ascontiguousarray(
            wg2m[dsl].reshape(GDT, 128, 128).transpose(1, 0, 2)
            .reshape(128, GDT * 128)).astype(BF)
        m["bg1"] = np.ascontiguousarray(
            bg1[dsl].reshape(GDT, 128).T).astype(np.float32)
        m["bg1m"] = np.ascontiguousarray(
            bg1m[dsl].reshape(GDT, 128).T).astype(np.float32)
        in_maps.append(m)
    return in_maps


_NC = None


def _get_nc():
    global _NC
    if _NC is None:
        _NC = _build()
    return _NC


def _host_combine(outs):
    """outs: [8, 1, OUTW] per-core partial rows -> final scalar loss.

    Per core: [0:6272] Z row-sum partials over its 784 logit columns,
    [6272] partial sum(max sim) over its rows, [6273:6281] partial
    sum(exp(l_neg/tau)) per image over its queue shard, [6281:6289]
    l_pos per image (replicated).
    """
    outs = np.asarray(outs, np.float64).reshape(len(outs), -1)
    Zf = outs[:, 0:NT].sum(axis=0)
    possum = outs[:, NT].sum()
    l_d = np.mean(np.log(Zf)) - ISC * possum / NT
    qsums = outs[:, NT + 1:NT + 9].sum(axis=0)
    lpos = outs[0, NT + 9:NT + 17]
    lse = np.log(np.exp(ISC * lpos) + qsums)
    l_g = np.mean(lse - ISC * lpos)
    return np.float32((1.0 - LAM) * l_g + LAM * l_d).reshape(())


def kernel(**inputs) -> np.ndarray:
    nc = _get_nc()
    in_maps = _prep_inputs(inputs)
    res = bass_utils.run_bass_kernel_spmd(nc, in_maps,
                                          core_ids=list(range(N_CORES)))
    outs = np.stack([res.results[c]["out"].reshape(1, OUTW)
                     for c in range(N_CORES)])
    return _host_combine(outs)


# revision 33
# speedup vs baseline: 1.1737x; 1.0457x over previous
"""DenseCL loss kernel for 8 TRN2 NeuronCores.

Sharding: core c owns batch image c for the dense branch, queue rows
[c*8192, (c+1)*8192) for the queue-InfoNCE negatives, and the COLUMN block
[c*784, (c+1)*784) of the flat dense-InfoNCE logits.

Key identity: matched_k[j] = k_d[:, idx_j], so the dense logits matrix is a
column gather of P = k_d_local^T @ q_all.  Each core computes partial row
sums Z_i = sum_m c_m * exp(P[m, i] / tau) where c is the histogram of its
own argmax indices (the weighted partition sum runs on the PE with the
counts as a stationary column), and the positives are the sim row maxima.
No matched-key gather and no matched-key AllGather is needed.

Collectives (gpsimd stream): a dummy 32-byte AllGather issued first thing
absorbs the cross-core start-skew barrier; then AllGather of pooled
features, AllGather of normalized q_d (fp8 bytes moved as f32 elements,
hidden under the k branch), AllReduce of the D-sharded global-head
partials.  Final ~10K-flop unshard happens on the host.
"""
import os
import sys

if "/opt/trn_rl_repo" not in sys.path:
    sys.path.insert(0, "/opt/trn_rl_repo")

USE_DR = os.environ.get("KDR", "1") == "1"      # fp8 DoubleRow for dense L1

import numpy as np
import ml_dtypes

import concourse.bass as bass
import concourse.bacc as bacc
import concourse.mybir as mybir
import concourse.tile as tile
from concourse import bass_utils, masks
from concourse.tile import add_dep_helper

BF = ml_dtypes.bfloat16
F8NP = ml_dtypes.float8_e4m3
F32 = mybir.dt.float32
BF16 = mybir.dt.bfloat16
F8 = mybir.dt.float8e4
DR = mybir.MatmulPerfMode.DoubleRow

N_CORES = 8
B, HW, C, D, P, Q = 8, 784, 1024, 2048, 128, 65536
QSH = Q // N_CORES          # 8192 queue rows per core
CT, DT = C // 128, D // 128  # 8, 16
GDT = DT // N_CORES         # 2 ghead D-tiles per core
NT = B * HW                 # 6272 total dense rows
TAU = 0.2
LAM = 0.5
ISC = 1.0 / TAU             # 5.0
WSCALE = 32.0               # fp8 range scale for W1/b1 (cancelled by l2 norm)
AF = mybir.ActivationFunctionType
ALU = mybir.AluOpType

# 784 = 6*128 + 16 partition tiles
PT = [(i * 128, min(128, HW - i * 128)) for i in range(7)]
OUTW = 8192                 # out row: [0:6272] Z, 6272 possum,
                            # [6273:6281] qsums, [6281:6289] lpos


def _chunks(n, step=512):
    return [(o, min(step, n - o)) for o in range(0, n, step)]


def _patch_act_tables():
    """Force every activation we use onto the natural_log_exp_and_others
    table set so the kernel needs exactly one ACT_TABLE_LOAD."""
    from concourse import hw_specs
    import concourse.bacc as bacc_mod
    if getattr(bacc_mod, "_act_tables_patched", False):
        return
    orig = hw_specs.get_activation_tables
    ours = {AF.Exp, AF.Ln, AF.Relu, AF.Identity, AF.Copy, AF.Square}
    keep = "natural_log_exp_and_others"

    def patched(arch):
        tabs = orig(arch)
        assert keep in tabs and ours <= tabs[keep]
        return {name: (fns if name == keep else fns - ours)
                for name, fns in tabs.items()}

    bacc_mod.get_activation_tables = patched
    bacc_mod._act_tables_patched = True


def _build(do_compile=True):
    _patch_act_tables()
    nc = bacc.Bacc("TRN2", target_bir_lowering=False, debug=False,
                   num_devices=N_CORES)

    def inp(name, shape, dt):
        return nc.dram_tensor(name, list(shape), dt, kind="ExternalInput")

    xq_d = inp("xq", (128, CT * HW), F8)          # [c, ct*784+p] = feat_q[b, p, ct*128+c]
    xk_d = inp("xk", (128, CT * HW), F8)
    wd1_d = inp("wd1", (DT, 128, C), F8)          # [dt, c, ct*128+d] = 32*Wd1[ct*128+c, dt*128+d]
    wd1m_d = inp("wd1m", (DT, 128, C), F8)
    wd2_d = inp("wd2", (128, D), BF16)            # [c, dt*128+d] = Wd2[dt*128+c, d]
    wd2m_d = inp("wd2m", (128, D), BF16)
    wg1_d = inp("wg1", (128, CT * GDT * 128), BF16)  # per-core D-slice of Wg1
    wg1m_d = inp("wg1m", (128, CT * GDT * 128), BF16)
    wg2_d = inp("wg2", (128, GDT * 128), BF16)    # per-core D-slice of Wg2 (lhsT)
    wg2m_d = inp("wg2m", (128, GDT * 128), BF16)
    bd1_d = inp("bd1", (128, DT), F32)            # [r, dt] = 32*bd1[dt*128+r]
    bd1m_d = inp("bd1m", (128, DT), F32)
    bd2_d = inp("bd2", (128, 1), F32)             # 32*bd2
    bd2m_d = inp("bd2m", (128, 1), F32)
    bg1_d = inp("bg1", (128, GDT), F32)           # per-core D-slice of bg1
    bg1m_d = inp("bg1m", (128, GDT), F32)
    bg2_d = inp("bg2", (128, 1), F32)
    bg2m_d = inp("bg2m", (128, 1), F32)
    queueT_d = inp("queueT", (128, QSH), F8)      # [ch, j] = queue[c0+j, ch]
    iotap_d = inp("iotap", (128, 8), F32)         # col i = p + 128*i
    onesc_d = inp("onesc", (128, 1), F32)         # ones column (lhsT partition sums)
    onesr_d = inp("onesr", (1, 128), F32)         # ones row (lhsT for K=1 broadcast)

    out_d = nc.dram_tensor("out", [1, OUTW], F32, kind="ExternalOutput")

    with tile.TileContext(nc) as tc:
        rg = [list(range(N_CORES))]
        with tc.tile_pool(name="dramp", bufs=1, space="DRAM") as dpool:
            pool_in = dpool.tile([2 * C], F32, name="pool_in")
            pool_out = dpool.tile([N_CORES * 2 * C], F32, name="pool_out",
                                  addr_space="Shared")
            # q_d fp8 bytes shipped as f32 elements (4x fewer CCE elements)
            qd_in = dpool.tile([128 * HW // 4], F32, name="qd_in")
            qd_out = dpool.tile([N_CORES * 128 * HW // 4], F32, name="qd_out",
                                addr_space="Shared")
            gar_in = dpool.tile([128 * 16], F32, name="gar_in")
            gar_out = dpool.tile([128 * 16], F32, name="gar_out",
                                 addr_space="Shared")
            _body(nc, tc, rg, locals())
    if do_compile:
        nc.compile()
    return nc


def _body(nc, tc, rg, env):
    g = lambda k: env[k]

    with tc.tile_pool(name="cst", bufs=1) as cst:

        def load(name, shape, dt, eng=None):
            t = cst.tile(list(shape), dt, name=name + "_sb")
            (eng or nc.sync).dma_start(t[:], g(name + "_d")[:])
            return t

        iotap_sb = load("iotap", (128, 8), F32, eng=nc.gpsimd)
        onesc_sb = load("onesc", (128, 1), F32, eng=nc.gpsimd)
        onesr_sb = load("onesr", (1, 128), F32, eng=nc.gpsimd)

        # ---- sync ring: q-branch critical inputs (per-dt weight slices)
        bd1_sb = load("bd1", (128, DT), F32)
        bd2_sb = load("bd2", (128, 1), F32)
        xq_sb = cst.tile([128, CT * HW], F8, name="xq_sb")
        nc.sync.dma_start(xq_sb[:, 0:4 * HW], g("xq_d")[:, 0:4 * HW])
        wq1_sb = cst.tile([128, DT * C], F8, name="wq1_sb")
        wd2_sb = cst.tile([128, D], BF16, name="wd2_sb")
        for dt in range(DT):
            nc.sync.dma_start(wq1_sb[:, dt * C:(dt + 1) * C],
                              g("wd1_d")[dt, :, :])
            if dt == 0:
                nc.sync.dma_start(xq_sb[:, 4 * HW:CT * HW],
                                  g("xq_d")[:, 4 * HW:CT * HW])
            if dt == 2:
                nc.sync.dma_start(wd2_sb[:], g("wd2_d")[:])
        bd1m_sb = load("bd1m", (128, DT), F32)
        bd2m_sb = load("bd2m", (128, 1), F32)

        # ---- gpsimd (SWDGE) ring: k-branch + tail inputs, so the scalar
        # queue carries only ACT work (DMA triggers head-of-line-block an
        # engine queue once the ring fills)
        xk_sb = cst.tile([128, CT * HW], F8, name="xk_sb")
        nc.gpsimd.dma_start(xk_sb[:], g("xk_d")[:])
        wk1_sb = cst.tile([128, DT * C], F8, name="wk1_sb")
        for dt in range(DT):
            nc.gpsimd.dma_start(wk1_sb[:, dt * C:(dt + 1) * C],
                                g("wd1m_d")[dt, :, :])
        wd2m_sb = load("wd2m", (128, D), BF16, eng=nc.gpsimd)

        onescb_sb = cst.tile([128, 1], BF16, name="onescb_sb")
        nc.vector.tensor_copy(onescb_sb[:], onesc_sb[:])
        id_f = cst.tile([128, 128], F32, name="id_f")
        masks.make_identity(nc, id_f[:])

        # long-lived results
        qdT_bf = cst.tile([128, HW], BF16, name="qdT_bf")
        kdT_bf = cst.tile([128, HW], BF16, name="kdT_bf")
        kdT_f8 = cst.tile([128, HW], F8, name="kdT_f8")
        qdT_f8 = cst.tile([128, HW], F8, name="qdT_f8")
        qall_sb = cst.tile([128, NT], F8, name="qall_sb")
        qgT_bf = cst.tile([128, 8], BF16, name="qgT_bf")
        kgT_bf = cst.tile([128, 8], BF16, name="kgT_bf")
        qgT_f8 = cst.tile([128, 8], F8, name="qgT_f8")
        pool_sb = cst.tile([128, 16], F32, name="pool_sb")
        gqall = cst.tile([128, 64], F32, name="gqall")  # pooled q [c, (r t)]
        gkall = cst.tile([128, 64], F32, name="gkall")
        fin_sb = cst.tile([1, 16], F32, name="fin_sb")

        ctx = dict(locals())
        _dense(nc, tc, rg, env, cst, ctx)
        _tail(nc, tc, rg, env, cst, ctx)


def _dense(nc, tc, rg, env, cst, ctx):
    g = lambda k: env[k]
    c = lambda k: ctx[k]
    pool_in, pool_out = g("pool_in"), g("pool_out")
    qd_in, qd_out = g("qd_in"), g("qd_out")
    xq_sb, xk_sb = c("xq_sb"), c("xk_sb")
    pool_sb = c("pool_sb")

    with tc.tile_pool(name="hp", bufs=3) as hp, \
         tc.tile_pool(name="l2s", bufs=2) as l2s, \
         tc.tile_pool(name="plp", bufs=2) as plp, \
         tc.tile_pool(name="ps_big", bufs=2, space="PSUM") as ps_big, \
         tc.tile_pool(name="ps_qd", bufs=2, space="PSUM") as ps_qd:

        def dense_branch(br, xs, w1sb, w2sb, b1, b2, dst, dst8):
            qd_ps = ps_qd.tile([128, HW], F32, name="qd_ps", tag="qd")
            for dt in range(DT):
                w1t = w1sb[:, dt * C:(dt + 1) * C]
                h_ps = ps_big.tile([128, HW], F32, name="h_ps", tag="big")
                if USE_DR:
                    for cp in range(CT // 2):
                        wp = w1t[:, cp * 256:(cp + 1) * 256].rearrange(
                            "p (two m) -> p two m", two=2)
                        xp = xs[:, cp * 2 * HW:(cp + 1) * 2 * HW].rearrange(
                            "p (two n) -> p two n", two=2)
                        for (o, n) in _chunks(HW):
                            nc.tensor.matmul(
                                h_ps[:, o:o + n],
                                lhsT=wp,
                                rhs=xp[:, :, o:o + n],
                                start=(cp == 0), stop=(cp == CT // 2 - 1),
                                perf_mode=DR)
                else:
                    for ct in range(CT):
                        for (o, n) in _chunks(HW):
                            nc.tensor.matmul(
                                h_ps[:, o:o + n],
                                lhsT=w1t[:, ct * 128:(ct + 1) * 128],
                                rhs=xs[:, ct * HW + o:ct * HW + o + n],
                                start=(ct == 0), stop=(ct == CT - 1))
                h_sb = hp.tile([128, HW], BF16, name="h_sb")
                nc.scalar.activation(h_sb[:], h_ps[:], AF.Relu,
                                     bias=b1[:, dt:dt + 1])
                if br == 0 and dt in (2, 4, 6, 8):
                    # pooling of xq/xk on DVE while PE grinds L1
                    base = 0 if dt in (2, 4) else 8
                    src = xq_sb if dt in (2, 4) else xk_sb
                    c0 = 0 if dt in (2, 6) else 4
                    for ct2 in range(c0, c0 + 4):
                        scr = plp.tile([128, HW], BF16, name="pool_scr")
                        nc.vector.tensor_scalar(
                            scr[:], src[:, ct2 * HW:(ct2 + 1) * HW], 1.0,
                            None, op0=ALU.mult, op1=ALU.add,
                            accum_out=pool_sb[:, base + ct2:base + ct2 + 1])
                # L2 for the PREVIOUS dt: its relu finished during this
                # dt's L1 matmuls, so the PE never waits on the ACT chain
                if dt > 0:
                    for (o, n) in _chunks(HW):
                        nc.tensor.matmul(
                            qd_ps[:, o:o + n],
                            lhsT=w2sb[:, (dt - 1) * 128:dt * 128],
                            rhs=h_prev[:, o:o + n],
                            start=(dt == 1), stop=False)
                h_prev = h_sb
            for (o, n) in _chunks(HW):
                nc.tensor.matmul(
                    qd_ps[:, o:o + n],
                    lhsT=w2sb[:, (DT - 1) * 128:DT * 128],
                    rhs=h_prev[:, o:o + n],
                    start=False, stop=True)
            # bias + l2-normalize along channels (partition dim)
            qdT_f = l2s.tile([128, HW], F32, name="qdT_f")
            nc.scalar.activation(qdT_f[:], qd_ps[:], AF.Identity, bias=b2[:])
            sq = l2s.tile([128, HW], BF16, name="sq")
            nc.scalar.activation(sq[:], qdT_f[:], AF.Square)
            ssq_ps = ps_qd.tile([1, HW], F32, name="ssq_ps", tag="qd")
            for (o, n) in _chunks(HW):
                nc.tensor.matmul(ssq_ps[:, o:o + n], lhsT=c("onescb_sb")[:],
                                 rhs=sq[:, o:o + n], start=True, stop=True)
            nrm = l2s.tile([1, HW], F32, name="nrm")
            nc.vector.tensor_scalar_max(nrm[:], ssq_ps[:], 1e-12)
            # rsqrt(s) = exp(-0.5*ln(s)) keeps ACT on one table set
            nrm2 = l2s.tile([1, HW], F32, name="nrm2")
            nc.scalar.activation(nrm2[:], nrm[:], AF.Ln)
            rn = l2s.tile([1, HW], F32, name="rn")
            nc.scalar.activation(rn[:], nrm2[:], AF.Exp, scale=-0.5)
            rnb_ps = ps_qd.tile([128, HW], F32, name="rnb_ps", tag="qd")
            for (o, n) in _chunks(HW):
                nc.tensor.matmul(rnb_ps[:, o:o + n], lhsT=c("onesr_sb")[:],
                                 rhs=rn[:, o:o + n], start=True, stop=True)
            nc.vector.tensor_mul(dst[:], qdT_f[:], rnb_ps[:])
            ctx["last_dve_" + str(br)] = nc.vector.tensor_copy(dst8[:], dst[:])

        dense_branch(0, xq_sb, c("wq1_sb"), c("wd2_sb"), c("bd1_sb"),
                     c("bd2_sb"), c("qdT_bf"), c("qdT_f8"))
        # ship q_d: this collective gates the whole logits tail, so it
        # goes first on the collective stream
        nc.sync.dma_start(
            qd_in[:].rearrange("(c p) -> c p", c=128),
            c("qdT_f8")[:].bitcast(F32))
        nc.gpsimd.collective_compute(
            "AllGather", ALU.bypass, replica_groups=rg,
            ins=[qd_in.opt()], outs=[qd_out.opt()])
        # tail inputs ride the gpsimd ring while it is blocked on the AG
        queueT_sb = cst.tile([128, QSH], F8, name="queueT_sb")
        nc.gpsimd.dma_start(queueT_sb[:], g("queueT_d")[:])
        ctx["queueT_sb"] = queueT_sb
        for nm in ("bg1", "bg1m", "bg2", "bg2m"):
            t = cst.tile([128, GDT] if nm in ("bg1", "bg1m") else [128, 1],
                         F32, name=nm + "_sb")
            nc.gpsimd.dma_start(t[:], g(nm + "_d")[:])
            ctx[nm + "_sb"] = t
        for nm, w in (("wg1", CT * GDT * 128), ("wg1m", CT * GDT * 128),
                      ("wg2", GDT * 128), ("wg2m", GDT * 128)):
            t = cst.tile([128, w], BF16, name=nm + "_sb")
            nc.gpsimd.dma_start(t[:], g(nm + "_d")[:])
            ctx[nm + "_sb"] = t
        # pooled features AllGather (second on the stream)
        pin = pool_in[:].rearrange("(g t c) -> c (g t)", g=2, t=8, c=128)
        nc.gpsimd.dma_start(pin, pool_sb[:])
        nc.gpsimd.collective_compute(
            "AllGather", ALU.bypass, replica_groups=rg,
            ins=[pool_in.opt()], outs=[pool_out.opt()])
        # qall load split across the sync and tensor rings
        for r in range(8):
            qall_i = nc.sync.dma_start(
                c("qall_sb")[:, r * HW:(r + 1) * HW].bitcast(F32),
                qd_out[r * 128 * HW // 4:(r + 1) * 128 * HW // 4]
                .rearrange("(c p) -> c p", c=128))
        # pooled features for every image: [c, (r t)] layout; ordered
        # behind the qall slices on the sync ring
        pg = pool_out[:].rearrange("(r g x) -> r g x", r=8, g=2)
        first = True
        for gi, dstp in ((0, c("gqall")), (1, c("gkall"))):
            for r in range(8):
                gi_i = nc.sync.dma_start(
                    dstp[:, r * 8:(r + 1) * 8],
                    pg[r, gi, :].rearrange("(t c) -> c t", c=128))
                if first:
                    add_dep_helper(gi_i.ins, qall_i.ins, sync=False,
                                   reason="gq loads after qall slices")
                    first = False

        dense_branch(1, xk_sb, c("wk1_sb"), c("wd2m_sb"), c("bd1m_sb"),
                     c("bd2m_sb"), c("kdT_bf"), c("kdT_f8"))


def _tail(nc, tc, rg, env, cst, ctx):
    """Everything after the dense branches, emission-ordered so no engine
    queue ever head-of-line-blocks on late data:

      sim -> argmax(DVE, concurrent) -> P/exp chunks 0-3 -> histogram ->
      ghead partials + AllReduce -> P/exp chunks 4-7 -> Z (PE, chasing) ->
      qg norm + lpos + queue negatives.
    """
    g = lambda k: env[k]
    c = lambda k: ctx[k]
    out_d = g("out_d")
    gar_in, gar_out = g("gar_in"), g("gar_out")
    qdT_bf, kdT_bf = c("qdT_bf"), c("kdT_bf")
    qall_sb, kdT_f8 = c("qall_sb"), c("kdT_f8")
    fin_sb = c("fin_sb")
    qgT_bf, kgT_bf, qgT_f8 = c("qgT_bf"), c("kgT_bf"), c("qgT_f8")

    with tc.tile_pool(name="cor", bufs=1) as cor, \
         tc.tile_pool(name="corS", bufs=2) as corS, \
         tc.tile_pool(name="lg", bufs=1) as lgp, \
         tc.tile_pool(name="gh", bufs=1) as gh, \
         tc.tile_pool(name="ps_lg", bufs=2, space="PSUM") as ps_lg, \
         tc.tile_pool(name="ps_sim", bufs=1, space="PSUM") as ps_sim, \
         tc.tile_pool(name="ps_ir", bufs=1, space="PSUM") as ps_ir, \
         tc.tile_pool(name="ps_z", bufs=1, space="PSUM") as ps_z:

        # ---- 1. sim + argmax (argmax chain runs on DVE concurrent with
        # the P/exp chunks below)
        sim_sb = cor.tile([128, 7 * HW], BF16, name="sim_sb")
        mx8 = cor.tile([128, 8], BF16, name="mx8")
        ix8 = cor.tile([128, 8], mybir.dt.uint32, name="ix8")
        ixf = cor.tile([128, 7], F32, name="ixf")
        posv = cor.tile([128, 7], F32, name="posv")
        nc.vector.memset(posv[:], 0.0)
        for i, (po_, pn) in enumerate(PT):
            s_ps = ps_sim.tile([128, HW], F32, name="s_ps", tag="sim")
            for (o, n) in _chunks(HW):
                nc.tensor.matmul(s_ps[0:pn, o:o + n],
                                 lhsT=qdT_bf[:, po_:po_ + pn],
                                 rhs=kdT_bf[:, o:o + n],
                                 start=True, stop=True)
            nc.scalar.activation(sim_sb[0:pn, i * HW:i * HW + HW],
                                 s_ps[0:pn, :], AF.Copy)
            nc.vector.max(mx8[0:pn, :], sim_sb[0:pn, i * HW:i * HW + HW])
            nc.vector.max_index(ix8[0:pn, :], mx8[0:pn, :],
                                sim_sb[0:pn, i * HW:i * HW + HW])
            nc.vector.tensor_copy(ixf[0:pn, i:i + 1], ix8[0:pn, 0:1])
            nc.vector.tensor_copy(posv[0:pn, i:i + 1], mx8[0:pn, 0:1])

        # ---- 2. P/exp, per-rank chunks (column-sharded logits)
        E_all = lgp.tile([128, 7 * NT], BF16, name="E_all")
        cpartb = cor.tile([128, 7], BF16, name="cpartb")

        def pexp(r):
            co = r * HW
            for j, (po_, pn) in enumerate(PT):
                lg_ps = ps_lg.tile([128, HW], F32, name="lg_ps")
                for (o, n) in _chunks(HW):
                    mm_i = nc.tensor.matmul(
                        lg_ps[0:pn, o:o + n],
                        lhsT=kdT_f8[:, po_:po_ + pn],
                        rhs=qall_sb[:, co + o:co + o + n],
                        start=True, stop=True)
                ex_i = nc.scalar.activation(
                    E_all[0:pn, j * NT + co:j * NT + co + HW],
                    lg_ps[0:pn, :], AF.Exp, scale=ISC)
            return mm_i, ex_i

        for r in range(4):
            p3_mm, p3_ex = pexp(r)

        # ---- 3. histogram of argmax indices + positives partial
        ir_sb = cor.tile([1, HW], F32, name="ir_sb")
        for i, (po_, pn) in enumerate(PT):
            ir_ps = ps_ir.tile([1, 128], F32, name="ir_ps", tag="ir")
            nc.tensor.transpose(ir_ps[0:1, 0:pn], ixf[0:pn, i:i + 1],
                                c("id_f")[0:pn, 0:pn])
            nc.scalar.activation(ir_sb[0:1, po_:po_ + pn],
                                 ir_ps[0:1, 0:pn], AF.Copy)
        ib_ps = ps_sim.tile([128, HW], F32, name="ib_ps", tag="sim")
        for (o, n) in _chunks(HW):
            nc.tensor.matmul(ib_ps[:, o:o + n], lhsT=c("onesr_sb")[:],
                             rhs=ir_sb[:, o:o + n], start=True, stop=True)
        ib_sb = cor.tile([128, HW], F32, name="ib_sb")
        nc.scalar.activation(ib_sb[:], ib_ps[:], AF.Copy)
        cpart = cor.tile([128, 7], F32, name="cpart")
        nc.vector.memset(cpart[:], 0.0)
        for i, (po_, pn) in enumerate(PT):
            S = corS.tile([128, HW], BF16, name="S")
            nc.vector.tensor_scalar(
                S[0:pn, :], ib_sb[0:pn, :], c("iotap_sb")[0:pn, i:i + 1],
                None, op0=ALU.is_equal, op1=ALU.add,
                accum_out=cpart[0:pn, i:i + 1])
        cpartb_i = nc.vector.tensor_copy(cpartb[:], cpart[:])
        pos_ps = ps_ir.tile([1, 128], F32, name="pos_ps", tag="ir")
        nc.tensor.matmul(pos_ps[0:1, 0:7], lhsT=c("onesc_sb")[:],
                         rhs=posv[:], start=True, stop=True)
        nc.vector.reduce_sum(fin_sb[0:1, 0:1], pos_ps[0:1, 0:7],
                             axis=mybir.AxisListType.X)
        nc.sync.dma_start(out_d[0:1, NT:NT + 1], fin_sb[0:1, 0:1])

        # ---- 4. global-head partials (D-sharded) + AllReduce
        gq_bf = gh.tile([128, 64], BF16, name="gq_bf")
        gk_bf = gh.tile([128, 64], BF16, name="gk_bf")
        gqm_i = nc.vector.tensor_scalar_mul(gq_bf[:], c("gqall")[:], 1.0 / HW)
        add_dep_helper(gqm_i.ins, cpartb_i.ins, sync=False,
                       reason="ghead DVE work after the argmax/hist chain")
        nc.vector.tensor_scalar_mul(gk_bf[:], c("gkall")[:], 1.0 / HW)
        gprt = gh.tile([128, 16], F32, name="gprt")
        for br2, (gsb, w1sb, w2sb, b1c) in enumerate([
                (gq_bf, c("wg1_sb"), c("wg2_sb"), c("bg1_sb")),
                (gk_bf, c("wg1m_sb"), c("wg2m_sb"), c("bg1m_sb"))]):
            gv = gsb[:].rearrange("c (r t) -> c t r", t=8)
            qg_ps = ps_z.tile([128, 8], F32, name="qg_ps", tag="z")
            for dl in range(GDT):
                hgt_ps = ps_ir.tile([128, 8], F32, name="hgt_ps", tag="ir")
                for ct in range(CT):
                    gh_mm = nc.tensor.matmul(
                        hgt_ps[:],
                        lhsT=w1sb[:, (ct * GDT + dl) * 128:
                                  (ct * GDT + dl + 1) * 128],
                        rhs=gv[:, ct, :],
                        start=(ct == 0), stop=(ct == CT - 1))
                    if br2 == 0 and dl == 0 and ct == 0:
                        add_dep_helper(gh_mm.ins, p3_mm.ins, sync=False,
                                       reason="ghead MMs after P chunk 3")
                hgt_sb = gh.tile([128, 8], BF16, name=f"hgt{br2}_{dl}")
                nc.vector.tensor_scalar(hgt_sb[:], hgt_ps[:],
                                        b1c[:, dl:dl + 1], 0.0,
                                        op0=ALU.add, op1=ALU.max)
                nc.tensor.matmul(qg_ps[:],
                                 lhsT=w2sb[:, dl * 128:(dl + 1) * 128],
                                 rhs=hgt_sb[:], start=(dl == 0),
                                 stop=(dl == GDT - 1))
            nc.vector.tensor_copy(gprt[:, br2 * 8:br2 * 8 + 8], qg_ps[:])
        nc.gpsimd.dma_start(
            gar_in[:].rearrange("(c p) -> c p", c=128), gprt[:])
        nc.gpsimd.collective_compute(
            "AllReduce", ALU.add, replica_groups=rg,
            ins=[gar_in.opt()], outs=[gar_out.opt()])
        garT = gh.tile([128, 16], F32, name="garT")
        nc.gpsimd.dma_start(
            garT[:], gar_out[:].rearrange("(c p) -> c p", c=128))

        # ---- 5. remaining P/exp chunks
        for r in range(4, 8):
            p7_mm, p7_ex = pexp(r)

        # ---- 6. Z[i] = sum_m c_m E'[m, i] on the PE, chasing the exps
        zfirst = True
        for (co, cn) in _chunks(NT):
            zc_ps = ps_z.tile([1, 512], F32, name="zc_ps", tag="z")
            for j, (po_, pn) in enumerate(PT):
                z_mm = nc.tensor.matmul(
                    zc_ps[0:1, 0:cn],
                    lhsT=cpartb[0:pn, j:j + 1],
                    rhs=E_all[0:pn, j * NT + co:j * NT + co + cn],
                    start=(j == 0), stop=(j == 6))
                if zfirst:
                    add_dep_helper(z_mm.ins, p7_mm.ins, sync=False,
                                   reason="Z sums after the P stream")
                    zfirst = False
            zb = corS.tile([1, 512], F32, name="zb")
            nc.scalar.activation(zb[0:1, 0:cn], zc_ps[0:1, 0:cn], AF.Copy)
            nc.sync.dma_start(out_d[0:1, co:co + cn], zb[0:1, 0:cn])

        # ---- 7. qg/kg bias + l2 norm + lpos + queue negatives
        for br2, (b2, dstg) in enumerate([(c("bg2_sb"), qgT_bf),
                                          (c("bg2m_sb"), kgT_bf)]):
            qgT_f = gh.tile([128, 8], F32, name=f"qgT_f{br2}")
            nq_i = nc.scalar.activation(qgT_f[:], garT[:, br2 * 8:br2 * 8 + 8],
                                        AF.Identity, bias=b2[:])
            if br2 == 0:
                add_dep_helper(nq_i.ins, p7_ex.ins, sync=False,
                               reason="qg norm after the exp chain")
            sqg = gh.tile([128, 8], BF16, name=f"sqg{br2}")
            nc.scalar.activation(sqg[:], qgT_f[:], AF.Square)
            ssg_ps = ps_sim.tile([1, 8], F32, name="ssg_ps", tag="sim")
            nc.tensor.matmul(ssg_ps[:], lhsT=c("onescb_sb")[:],
                             rhs=sqg[:], start=True, stop=True)
            nrg = gh.tile([1, 8], F32, name=f"nrg{br2}")
            nc.vector.tensor_scalar_max(nrg[:], ssg_ps[:], 1e-12)
            nrg2 = gh.tile([1, 8], F32, name=f"nrg2{br2}")
            nc.scalar.activation(nrg2[:], nrg[:], AF.Ln)
            rng = gh.tile([1, 8], F32, name=f"rng{br2}")
            nc.scalar.activation(rng[:], nrg2[:], AF.Exp, scale=-0.5)
            rngb_ps = ps_sim.tile([128, 8], F32, name="rngb_ps", tag="sim")
            nc.tensor.matmul(rngb_ps[:], lhsT=c("onesr_sb")[:],
                             rhs=rng[:], start=True, stop=True)
            nc.vector.tensor_mul(dstg[:], qgT_f[:], rngb_ps[:])
        lpm = gh.tile([128, 8], F32, name="lpm")
        nc.vector.tensor_mul(lpm[:], qgT_bf[:], kgT_bf[:])
        lp_ps = ps_sim.tile([1, 8], F32, name="lp_ps", tag="sim")
        nc.tensor.matmul(lp_ps[:], lhsT=c("onesc_sb")[:], rhs=lpm[:],
                         start=True, stop=True)
        nc.scalar.activation(fin_sb[0:1, 1:9], lp_ps[:], AF.Copy)
        nc.sync.dma_start(out_d[0:1, NT + 9:NT + 17], fin_sb[0:1, 1:9])
        nc.vector.tensor_copy(qgT_f8[:], qgT_bf[:])
        # queue negatives: transposed orientation, single exp
        qe_ps = ps_z.tile([128, 512], F32, name="qe_ps", tag="z")
        for qt in range(64):
            qmm_i = nc.tensor.matmul(
                qe_ps[:, qt * 8:(qt + 1) * 8],
                lhsT=c("queueT_sb")[:, qt * 128:(qt + 1) * 128],
                rhs=qgT_f8[:], start=True, stop=True)
            if qt == 0:
                add_dep_helper(qmm_i.ins, z_mm.ins, sync=False,
                               reason="queue negatives after Z sums")
        qe_sb = gh.tile([128, 512], BF16, name="qe_sb")
        nc.scalar.activation(qe_sb[:], qe_ps[:], AF.Exp, scale=ISC)
        qs_ps = ps_sim.tile([1, 512], F32, name="qs_ps", tag="sim")
        for (o, n) in _chunks(512):
            nc.tensor.matmul(qs_ps[:, o:o + n], lhsT=c("onescb_sb")[:],
                             rhs=qe_sb[:, o:o + n], start=True, stop=True)
        qsum_r = gh.tile([1, 8], F32, name="qsum_r")
        nc.vector.reduce_sum(qsum_r[:],
                             qs_ps[:].rearrange("p (t i) -> p i t", i=8),
                             axis=mybir.AxisListType.X)
        nc.sync.dma_start(out_d[0:1, NT + 1:NT + 9], qsum_r[:])


def _prep_inputs(inputs):
    fq = np.asarray(inputs["feat_q"], np.float32).reshape(B, HW, C)
    fk = np.asarray(inputs["feat_k"], np.float32).reshape(B, HW, C)

    def xT(x):  # (784, 1024) -> (128, 8*784) fp8 with [c, ct*784+p]
        return np.ascontiguousarray(
            x.reshape(HW, CT, 128).transpose(2, 1, 0).reshape(128, CT * HW)
        ).astype(F8NP)

    def w1tile(w):  # (1024, 2048) -> (16, 128, 1024) fp8, scaled
        return np.ascontiguousarray(
            (w * WSCALE).reshape(CT, 128, DT, 128).transpose(2, 1, 0, 3)
            .reshape(DT, 128, C)).astype(F8NP)

    def w2tile(w):  # (2048, 128) -> (128, 2048) with [c, dt*128+d]
        return np.ascontiguousarray(
            w.reshape(DT, 128, 128).transpose(1, 0, 2).reshape(128, D)
        ).astype(BF)

    queue = np.asarray(inputs["queue"], np.float32)
    wg1 = np.asarray(inputs["Wg1"], np.float32)   # (1024, 2048)
    wg1m = np.asarray(inputs["mWg1"], np.float32)
    wg2 = np.asarray(inputs["Wg2"], np.float32)   # (2048, 128)
    wg2m = np.asarray(inputs["mWg2"], np.float32)
    bg1 = np.asarray(inputs["bg1"], np.float32)
    bg1m = np.asarray(inputs["mbg1"], np.float32)

    iotap = (np.arange(128, dtype=np.float32)[:, None]
             + 128.0 * np.arange(8, dtype=np.float32)[None, :])

    shared = {
        "wd1": w1tile(np.asarray(inputs["Wd1"], np.float32)),
        "wd1m": w1tile(np.asarray(inputs["mWd1"], np.float32)),
        "wd2": w2tile(np.asarray(inputs["Wd2"], np.float32)),
        "wd2m": w2tile(np.asarray(inputs["mWd2"], np.float32)),
        "bd1": np.ascontiguousarray(
            (np.asarray(inputs["bd1"], np.float32) * WSCALE)
            .reshape(DT, 128).T).astype(np.float32),
        "bd1m": np.ascontiguousarray(
            (np.asarray(inputs["mbd1"], np.float32) * WSCALE)
            .reshape(DT, 128).T).astype(np.float32),
        "bd2": (np.asarray(inputs["bd2"], np.float32) * WSCALE
                ).reshape(128, 1),
        "bd2m": (np.asarray(inputs["mbd2"], np.float32) * WSCALE
                 ).reshape(128, 1),
        "bg2": np.asarray(inputs["bg2"], np.float32).reshape(128, 1),
        "bg2m": np.asarray(inputs["mbg2"], np.float32).reshape(128, 1),
        "iotap": np.ascontiguousarray(iotap),
        "onesc": np.ones((128, 1), np.float32),
        "onesr": np.ones((1, 128), np.float32),
    }
    in_maps = []
    for cc in range(N_CORES):
        m = dict(shared)
        m["xq"] = xT(fq[cc])
        m["xk"] = xT(fk[cc])
        m["queueT"] = np.ascontiguousarray(
            queue[cc * QSH:(cc + 1) * QSH].T).astype(F8NP)
        # per-core D-slice of the global head: dts {2c, 2c+1}
        dsl = slice(cc * GDT * 128, (cc + 1) * GDT * 128)
        # wg1 slice layout [c, (ct*GDT+dl)*128 + d]
        m["wg1"] = np.ascontiguousarray(
            wg1[:, dsl].reshape(CT, 128, GDT * 128).transpose(1, 0, 2)
            .reshape(128, CT * GDT * 128)).astype(BF)
        m["wg1m"] = np.ascontiguousarray(
            wg1m[:, dsl].reshape(CT, 128, GDT * 128).transpose(1, 0, 2)
            .reshape(128, CT * GDT * 128)).astype(BF)
        # wg2 slice [dl*128+r, P] -> lhsT layout [r, dl*128+p]
        m["wg2"] = np.ascontiguousarray(
            wg2[dsl].reshape(GDT, 128, 128).transpose(1, 0, 2)
            .reshape(128, GDT * 128)).astype(BF)
        m["wg2m"] = np.ascontiguousarray(
            wg2m[dsl].reshape(GDT, 128, 128).transpose(1, 0, 2)
            .reshape(128, GDT * 128)).astype(BF)
        m["bg1"] = np.ascontiguousarray(
            bg1[dsl].reshape(GDT, 128).T).astype(np.float32)
        m["bg1m"] = np.ascontiguousarray(
            bg1m[dsl].reshape(GDT, 128).T).astype(np.float32)
        in_maps.append(m)
    return in_maps


_NC = None


def _get_nc():
    global _NC
    if _NC is None:
        _NC = _build()
    return _NC


def _host_combine(outs):
    """outs: [8, 1, OUTW] per-core partial rows -> final scalar loss.

    Per core: [0:6272] Z row-sum partials over its 784 logit columns,
    [6272] partial sum(max sim) over its rows, [6273:6281] partial
    sum(exp(l_neg/tau)) per image over its queue shard, [6281:6289]
    l_pos per image (replicated).
    """
    outs = np.asarray(outs, np.float64).reshape(len(outs), -1)
    Zf = outs[:, 0:NT].sum(axis=0)
    possum = outs[:, NT].sum()
    l_d = np.mean(np.log(Zf)) - ISC * possum / NT
    qsums = outs[:, NT + 1:NT + 9].sum(axis=0)
    lpos = outs[0, NT + 9:NT + 17]
    lse = np.log(np.exp(ISC * lpos) + qsums)
    l_g = np.mean(lse - ISC * lpos)
    return np.float32((1.0 - LAM) * l_g + LAM * l_d).reshape(())


def kernel(**inputs) -> np.ndarray:
    nc = _get_nc()
    in_maps = _prep_inputs(inputs)
    res = bass_utils.run_bass_kernel_spmd(nc, in_maps,
                                          core_ids=list(range(N_CORES)))
    outs = np.stack([res.results[c]["out"].reshape(1, OUTW)
                     for c in range(N_CORES)])
    return _host_combine(outs)
